# revision 16
# baseline (speedup 1.0000x reference)
"""Causal self-attention (B=4, S=2048, D=1024, single head) on 8 TRN2 cores.

Sharding: data-parallel over batch (4 batches x 2 cores). The two cores of a
batch split the 16 query tiles (128 rows each) so both get exactly equal
causal work: core A takes tiles {0,2,4,6,9,11,13,15}, core B the complement.

v1 dataflow (default) eliminates the duplicated K/V projections entirely by
re-associating the attention algebra so that every projection-type matmul is
proportional to the LOCAL query count (1024 rows/core) instead of the full
key count (2048 keys, which both cores of a pair would otherwise each
project):

  scores = Q K^T = (Q Wk^T) X^T          -> Q'' = Q Wk^T is 1024-row local
  out    = P V   = (P X) Wv              -> P X is causal-sized; Wv-apply is
                                            1024-row local

Per core: Q^T projection (PE), Q''^T = Wk Q^T (PE), then per query-tile
slot: scores = Q''^T-lhsT vs X^T-rhs, exp on ACT with accumulated row sums,
P^T via plain matmuls against an identity rhs (cheaper than PE transpose
mode and HAM-warming), R^T = P X via pt-lhsT vs X-key-major rhs, R^T
re-transposed the same way, out = R^T Wv accumulated over d-chunks,
normalized by the reciprocal row sum. A block of warm-up matmuls on the
identity runs during the initial DMA window so the PE reaches its 2.4 GHz
clock (HAM K=8/8) before real work arrives.

Modes (BASS_KERNEL_MODE env, default "v1"):
  "v1"   — factored dataflow above; bf16 storage.
  "bf16" — previous baseline (duplicated K/V projections); ~233us.
"""

import os
from contextlib import ExitStack

import ml_dtypes
import numpy as np

import concourse.bacc as bacc
import concourse.mybir as mybir
import concourse.tile as tile
from concourse.bass_utils import run_bass_kernel_spmd

B, S, D = 4, 2048, 1024
P = 128
DC = D // P  # 8 contraction chunks
A_TILES = [0, 2, 4, 6, 9, 11, 13, 15]
B_TILES = [1, 3, 5, 7, 8, 10, 12, 14]
NS = [2, 4, 6, 8, 10, 12, 14, 16]  # key-tiles (128 keys) processed per slot
MASK_VAL = -60.0
N_WARM = 96

F32 = mybir.dt.float32
BF16 = mybir.dt.bfloat16

MODE = os.environ.get("BASS_KERNEL_MODE", "v1")

_compiled = {}


def _make_ident(tc, pool):
    nc = tc.nc
    ident = pool.tile([P, P], BF16, name="ident", tag="ident")
    nc.gpsimd.memset(ident[:], 1.0)
    nc.gpsimd.affine_select(
        out=ident[:],
        in_=ident[:],
        compare_op=mybir.AluOpType.is_equal,
        fill=0.0,
        base=0,
        pattern=[[-1, P]],
        channel_multiplier=1,
    )
    return ident


# ---------------------------------------------------------------------------
# v1: factored dataflow
# ---------------------------------------------------------------------------


def _build_v1():
    nc = bacc.Bacc("TRN2", target_bir_lowering=False, debug=False)
    xqT = nc.dram_tensor("xqT", [D, 1024], BF16, kind="ExternalInput").ap()
    xT = nc.dram_tensor("xT", [D, S], BF16, kind="ExternalInput").ap()
    xk = nc.dram_tensor("xk", [S, D], BF16, kind="ExternalInput").ap()
    wq = nc.dram_tensor("wq", [D, D], BF16, kind="ExternalInput").ap()
    wkT = nc.dram_tensor("wkT", [D, D], BF16, kind="ExternalInput").ap()
    wv = nc.dram_tensor("wv", [D, D], BF16, kind="ExternalInput").ap()
    msk = nc.dram_tensor("msk", [1024, 512], BF16, kind="ExternalInput").ap()
    out_d = nc.dram_tensor("out", [1024, D], F32, kind="ExternalOutput").ap()

    with tile.TileContext(nc) as tc:
        _body_v1(tc, xqT, xT, xk, wq, wkT, wv, msk, out_d)
    nc.compile()
    return nc


def _body_v1(tc, xqT, xT, xk, wq, wkT, wv, msk, out_d):
    nc = tc.nc
    with ExitStack() as top:
        const_pool = top.enter_context(tc.tile_pool(name="cst", bufs=1))
        ident = _make_ident(tc, const_pool)

        # whole-kernel residents
        res_pool = top.enter_context(tc.tile_pool(name="res", bufs=1))
        XT_t = [res_pool.tile([P, S], BF16, name=f"xt{d}", tag=f"xt{d}") for d in range(DC)]
        XK_t = [res_pool.tile([P, D], BF16, name=f"xk{k}", tag=f"xk{k}") for k in range(S // P)]
        Q2T = [res_pool.tile([P, 1024], BF16, name=f"q2t{d}", tag=f"q2t{d}") for d in range(DC)]
        wv_t = [res_pool.tile([P, D], BF16, name=f"wv{d}", tag=f"wv{d}") for d in range(DC)]

        # ---------------- projections ----------------
        with ExitStack() as ph:
            wq_pool = ph.enter_context(tc.tile_pool(name="wqp", bufs=1))
            xq_pool = ph.enter_context(tc.tile_pool(name="xqp", bufs=1))
            wkT_pool = ph.enter_context(tc.tile_pool(name="wkp", bufs=1))
            qt_pool = ph.enter_context(tc.tile_pool(name="qtp", bufs=1))
            pps = ph.enter_context(tc.tile_pool(name="pps", bufs=1, space="PSUM"))

            wq_t = [wq_pool.tile([P, D], BF16, name=f"wqt{d}", tag=f"wqt{d}") for d in range(DC)]
            xq_t = [xq_pool.tile([P, 1024], BF16, name=f"xq{d}", tag=f"xq{d}") for d in range(DC)]
            wkT_t = [wkT_pool.tile([P, D], BF16, name=f"wkt{e}", tag=f"wkt{e}") for e in range(DC)]
            QT = [qt_pool.tile([P, 1024], BF16, name=f"qt{e}", tag=f"qt{e}") for e in range(DC)]

            # DMA issue order = priority. Q-proj runs d-outer, so strip
            # PAIR d (wq_t[d] + xq_t[d], 0.5MB) is all the first d-sweep
            # needs — the PE starts ~3us in instead of waiting for 4MB.
            for d in range(DC):
                nc.sync.dma_start(wq_t[d][:], wq[d * P : (d + 1) * P, :])
                nc.sync.dma_start(xq_t[d][:], xqT[d * P : (d + 1) * P, :])
            for e in range(DC):
                nc.sync.dma_start(wkT_t[e][:], wkT[e * P : (e + 1) * P, :])
            for d in range(DC):
                nc.sync.dma_start(XT_t[d][:], xT[d * P : (d + 1) * P, :])
            for k in range(S // P):
                nc.sync.dma_start(XK_t[k][:], xk[k * P : (k + 1) * P, :])
            for d in range(DC):
                nc.sync.dma_start(wv_t[d][:], wv[d * P : (d + 1) * P, :])

            # Q^T projection, d-outer: 8 live psums (one per e-chunk) so the
            # d=0 sweep starts as soon as the first strip pair lands.
            for qc in range(2):
                qps = [pps.tile([P, 512], F32, name="pps", tag=f"pps{e}") for e in range(DC)]
                for d in range(DC):
                    for e in range(DC):
                        nc.tensor.matmul(
                            qps[e][:],
                            lhsT=wq_t[d][:, e * P : (e + 1) * P],
                            rhs=xq_t[d][:, qc * 512 : (qc + 1) * 512],
                            start=(d == 0),
                            stop=(d == DC - 1),
                        )
                for e in range(DC):
                    nc.scalar.copy(QT[e][:, qc * 512 : (qc + 1) * 512], qps[e][:])

            # Q''^T = Wk Q^T:  Q2T[dc][:, qc] = sum_e WkT[e, dc-chunk]^T QT[e, qc]
            for qc in range(2):
                for dc in range(DC):
                    ps = pps.tile([P, 512], F32, name="pps", tag=f"pps{dc}")
                    for e in range(DC):
                        nc.tensor.matmul(
                            ps[:],
                            lhsT=wkT_t[e][:, dc * P : (dc + 1) * P],
                            rhs=QT[e][:, qc * 512 : (qc + 1) * 512],
                            start=(e == 0),
                            stop=(e == DC - 1),
                        )
                    nc.scalar.copy(Q2T[dc][:, qc * 512 : (qc + 1) * 512], ps[:])

        # ---------------- attention over slots ----------------
        with ExitStack() as ph:
            m_pool = ph.enter_context(tc.tile_pool(name="mp", bufs=2))
            sm_pool = ph.enter_context(tc.tile_pool(name="smp", bufs=2))
            p_pool = ph.enter_context(tc.tile_pool(name="pp", bufs=2))
            pt_pool = ph.enter_context(tc.tile_pool(name="ptp", bufs=2))
            rt_pool = ph.enter_context(tc.tile_pool(name="rtp", bufs=2))
            r2_pool = ph.enter_context(tc.tile_pool(name="r2p", bufs=2))
            o_pool = ph.enter_context(tc.tile_pool(name="op", bufs=2))
            small_pool = ph.enter_context(tc.tile_pool(name="smallp", bufs=2))
            qk_ps = ph.enter_context(tc.tile_pool(name="qkps", bufs=2, space="PSUM"))
            tp_ps = ph.enter_context(tc.tile_pool(name="tpps", bufs=2, space="PSUM"))
            r_ps = ph.enter_context(tc.tile_pool(name="rps", bufs=2, space="PSUM"))

            state = {}

            def scores_part(s):
                n = NS[s]
                kw = n * P
                nchunks = (kw + 511) // 512
                mt = m_pool.tile([P, 512], BF16, name="mt", tag="mt")
                last_nw = kw - (nchunks - 1) * 512
                nc.sync.dma_start(mt[:, :last_nw], msk[s * P : (s + 1) * P, :last_nw])
                pb = p_pool.tile([P, S], BF16, name="pb", tag="pb")
                lparts = small_pool.tile([P, 4], F32, name="lparts", tag="lparts")
                for c in range(nchunks):
                    nw = min(512, kw - c * 512)
                    ps = qk_ps.tile([P, 512], F32, name="qk_t", tag="qk_t")
                    for dc in range(DC):
                        nc.tensor.matmul(
                            ps[:, :nw],
                            lhsT=Q2T[dc][:, s * P : (s + 1) * P],
                            rhs=XT_t[dc][:, c * 512 : c * 512 + nw],
                            start=(dc == 0),
                            stop=(dc == DC - 1),
                        )
                    if c == nchunks - 1:
                        sm = sm_pool.tile([P, 512], F32, name="sm", tag="sm")
                        nc.vector.tensor_add(sm[:, :nw], ps[:, :nw], mt[:, :nw])
                        src = sm[:, :nw]
                    else:
                        src = ps[:, :nw]
                    nc.scalar.activation(
                        pb[:, c * 512 : c * 512 + nw],
                        src,
                        mybir.ActivationFunctionType.Exp,
                        accum_out=lparts[:, c : c + 1],
                    )
                lsum = small_pool.tile([P, 1], F32, name="lsum", tag="lsum")
                nc.vector.reduce_sum(lsum[:], lparts[:, :nchunks], axis=mybir.AxisListType.X)
                rl = small_pool.tile([P, 1], F32, name="rl", tag="rl")
                nc.vector.reciprocal(rl[:], lsum[:])
                state[s] = (pb, rl)

            def tail_a(s):
                n = NS[s]
                pb, rl = state[s]
                # P^T via plain matmuls: out = pb_block^T @ I, 4 tiles per
                # PSUM bank, one ACT copy per bank.
                pt = pt_pool.tile([P, S], BF16, name="ptt", tag="ptt")
                for j0 in range(0, n, 4):
                    jn = min(4, n - j0)
                    tp = tp_ps.tile([P, 512], F32, name="tp_t", tag="tp_t")
                    for j in range(j0, j0 + jn):
                        nc.tensor.matmul(
                            tp[:, (j - j0) * P : (j - j0 + 1) * P],
                            lhsT=pb[:, j * P : (j + 1) * P],
                            rhs=ident[:],
                            start=True,
                            stop=True,
                        )
                    nc.vector.tensor_copy(
                        pt[:, j0 * P : (j0 + jn) * P], tp[:, : jn * P]
                    )
                # R^T = P X  ([128q, 1024d], accumulated over n key tiles)
                rp = r_ps.tile([P, 1024], F32, name="rp", tag="rp")
                for j in range(n):
                    for h in range(2):
                        nc.tensor.matmul(
                            rp[:, h * 512 : (h + 1) * 512],
                            lhsT=pt[:, j * P : (j + 1) * P],
                            rhs=XK_t[j][:, h * 512 : (h + 1) * 512],
                            start=(j == 0),
                            stop=(j == n - 1),
                        )
                rt = rt_pool.tile([P, 1024], BF16, name="rt", tag="rt")
                for h in range(2):
                    nc.vector.tensor_copy(rt[:, h * 512 : (h + 1) * 512], rp[:, h * 512 : (h + 1) * 512])
                state[s] = (state[s][0], state[s][1], rt)

            def tail_b(s):
                pb, rl, rt = state.pop(s)
                # transpose R^T -> R chunks [128d, 128q]
                r2 = r2_pool.tile([P, 1024], BF16, name="r2", tag="r2")
                for d0 in range(0, DC, 4):
                    tp = tp_ps.tile([P, 512], F32, name="tp_t", tag="tp_t")
                    for dc in range(d0, d0 + 4):
                        nc.tensor.matmul(
                            tp[:, (dc - d0) * P : (dc - d0 + 1) * P],
                            lhsT=rt[:, dc * P : (dc + 1) * P],
                            rhs=ident[:],
                            start=True,
                            stop=True,
                        )
                    nc.vector.tensor_copy(r2[:, d0 * P : (d0 + 4) * P], tp[:])
                # out = R^T Wv  (accumulate over the 8 d-chunks)
                ops = [
                    qk_ps.tile([P, 512], F32, name="qk_t", tag="qk_t"),
                    qk_ps.tile([P, 512], F32, name="qk_t", tag="qk_t"),
                ]
                for dc in range(DC):
                    for h in range(2):
                        nc.tensor.matmul(
                            ops[h][:],
                            lhsT=r2[:, dc * P : (dc + 1) * P],
                            rhs=wv_t[dc][:, h * 512 : (h + 1) * 512],
                            start=(dc == 0),
                            stop=(dc == DC - 1),
                        )
                ot = o_pool.tile([P, D], F32, name="ot", tag="ot")
                for h in range(2):
                    nc.vector.tensor_scalar_mul(ot[:, h * 512 : (h + 1) * 512], ops[h][:], rl[:])
                    nc.sync.dma_start(
                        out_d[s * P : (s + 1) * P, h * 512 : (h + 1) * 512],
                        ot[:, h * 512 : (h + 1) * 512],
                    )

            # one-slot software pipeline: PE runs tail(s-1) while ACT
            # computes exp(s). The last two slots' tail halves interleave
            # so slot 7's handoffs (exp -> PT, rp -> rt) stay covered.
            for s in range(8):
                scores_part(s)
                if 1 <= s <= 6:
                    tail_a(s - 1)
                    tail_b(s - 1)
            tail_a(6)
            tail_a(7)
            tail_b(6)
            tail_b(7)


# ---------------------------------------------------------------------------
# previous baseline (duplicated K/V projections) — kept for A/B comparison
# ---------------------------------------------------------------------------


def _build_bf16():
    nc = bacc.Bacc("TRN2", target_bir_lowering=False, debug=False)
    xqT = nc.dram_tensor("xqT", [D, 1024], BF16, kind="ExternalInput").ap()
    xT = nc.dram_tensor("xT", [D, S], BF16, kind="ExternalInput").ap()
    wq = nc.dram_tensor("wq", [D, D], BF16, kind="ExternalInput").ap()
    wk = nc.dram_tensor("wk", [D, D], BF16, kind="ExternalInput").ap()
    wv = nc.dram_tensor("wv", [D, D], BF16, kind="ExternalInput").ap()
    msk = nc.dram_tensor("msk", [1024, 512], F32, kind="ExternalInput").ap()
    out_d = nc.dram_tensor("out", [1024, D], F32, kind="ExternalOutput").ap()

    with tile.TileContext(nc) as tc:
        _body_bf16(tc, xqT, xT, wq, wk, wv, msk, out_d)
    nc.compile()
    return nc


def _body_bf16(tc, xqT, xT, wq, wk, wv, msk, out_d):
    nc = tc.nc
    with ExitStack() as top:
        const_pool = top.enter_context(tc.tile_pool(name="cst", bufs=1))
        ident = _make_ident(tc, const_pool)

        res_pool = top.enter_context(tc.tile_pool(name="res", bufs=1))
        QT = [res_pool.tile([P, 1024], BF16, name=f"qt{e}", tag=f"qt{e}") for e in range(DC)]
        KT = [res_pool.tile([P, S], BF16, name=f"kt{e}", tag=f"kt{e}") for e in range(DC)]
        V = [res_pool.tile([P, D], BF16, name=f"v{k}", tag=f"v{k}") for k in range(S // P)]

        w_pool = top.enter_context(tc.tile_pool(name="wp", bufs=1))
        wq_t = [w_pool.tile([P, D], BF16, name=f"wqt{d}", tag=f"wqt{d}") for d in range(DC)]
        wk_t = [w_pool.tile([P, D], BF16, name=f"wkt{d}", tag=f"wkt{d}") for d in range(DC)]
        wv_t = [w_pool.tile([P, D], BF16, name=f"wvt{d}", tag=f"wvt{d}") for d in range(DC)]

        psum = top.enter_context(tc.tile_pool(name="psum", bufs=2, space="PSUM"))
        xs_pool = top.enter_context(tc.tile_pool(name="xsp", bufs=2))

        xs0 = [xs_pool.tile([P, 512], BF16, name=f"xs{d}", tag=f"xs{d}") for d in range(DC)]
        for d in range(DC):
            nc.sync.dma_start(xs0[d][:], xT[d * P : (d + 1) * P, 0:512])
        for ec in range(2):
            for d in range(DC):
                nc.sync.dma_start(
                    wv_t[d][:, ec * 512 : (ec + 1) * 512],
                    wv[d * P : (d + 1) * P, ec * 512 : (ec + 1) * 512],
                )
        for d in range(DC):
            nc.sync.dma_start(wk_t[d][:], wk[d * P : (d + 1) * P, :])
        for kc in range(4):
            if kc == 0:
                xs = xs0
            else:
                xs = [xs_pool.tile([P, 512], BF16, name=f"xs{d}", tag=f"xs{d}") for d in range(DC)]
                for d in range(DC):
                    nc.sync.dma_start(xs[d][:], xT[d * P : (d + 1) * P, kc * 512 : (kc + 1) * 512])
            for j in range(4):
                kt_idx = kc * 4 + j
                for ec in range(2):
                    ps = psum.tile([P, 512], F32, name="pps", tag="pps")
                    for d in range(DC):
                        nc.tensor.matmul(
                            ps[:],
                            lhsT=xs[d][:, j * P : (j + 1) * P],
                            rhs=wv_t[d][:, ec * 512 : (ec + 1) * 512],
                            start=(d == 0),
                            stop=(d == DC - 1),
                        )
                    nc.scalar.copy(V[kt_idx][:, ec * 512 : (ec + 1) * 512], ps[:])
            for e in range(DC):
                ps = psum.tile([P, 512], F32, name="pps", tag="pps")
                for d in range(DC):
                    nc.tensor.matmul(
                        ps[:],
                        lhsT=wk_t[d][:, e * P : (e + 1) * P],
                        rhs=xs[d][:],
                        start=(d == 0),
                        stop=(d == DC - 1),
                    )
                nc.scalar.copy(KT[e][:, kc * 512 : (kc + 1) * 512], ps[:])

        for d in range(DC):
            nc.sync.dma_start(wq_t[d][:], wq[d * P : (d + 1) * P, :])
        for qc in range(2):
            xs = [xs_pool.tile([P, 512], BF16, name=f"xs{d}", tag=f"xs{d}") for d in range(DC)]
            for d in range(DC):
                nc.sync.dma_start(xs[d][:], xqT[d * P : (d + 1) * P, qc * 512 : (qc + 1) * 512])
            for e in range(DC):
                ps = psum.tile([P, 512], F32, name="pps", tag="pps")
                for d in range(DC):
                    nc.tensor.matmul(
                        ps[:],
                        lhsT=wq_t[d][:, e * P : (e + 1) * P],
                        rhs=xs[d][:],
                        start=(d == 0),
                        stop=(d == DC - 1),
                    )
                nc.scalar.copy(QT[e][:, qc * 512 : (qc + 1) * 512], ps[:])

        with ExitStack() as ph:
            m_pool = ph.enter_context(tc.tile_pool(name="mp", bufs=2))
            sm_pool = ph.enter_context(tc.tile_pool(name="smp", bufs=2))
            p_pool = ph.enter_context(tc.tile_pool(name="pp", bufs=2))
            pt_pool = ph.enter_context(tc.tile_pool(name="ptp", bufs=2))
            o_pool = ph.enter_context(tc.tile_pool(name="op", bufs=2))
            small_pool = ph.enter_context(tc.tile_pool(name="smallp", bufs=2))

            for s in range(8):
                n = NS[s]
                kw = n * P
                nchunks = (kw + 511) // 512
                mt = m_pool.tile([P, 512], F32, name="mt", tag="mt")
                last_nw = kw - (nchunks - 1) * 512
                nc.sync.dma_start(mt[:, :last_nw], msk[s * P : (s + 1) * P, :last_nw])
                pb = p_pool.tile([P, S], BF16, name="pb", tag="pb")
                lparts = small_pool.tile([P, 4], F32, name="lparts", tag="lparts")
                for c in range(nchunks):
                    nw = min(512, kw - c * 512)
                    ps = psum.tile([P, 512], F32, name="qk_t", tag="qk_t")
                    for e in range(DC):
                        nc.tensor.matmul(
                            ps[:, :nw],
                            lhsT=QT[e][:, s * P : (s + 1) * P],
                            rhs=KT[e][:, c * 512 : c * 512 + nw],
                            start=(e == 0),
                            stop=(e == DC - 1),
                        )
                    if c == nchunks - 1:
                        sm = sm_pool.tile([P, 512], F32, name="sm", tag="sm")
                        nc.vector.tensor_add(sm[:, :nw], ps[:, :nw], mt[:, :nw])
                        src = sm[:, :nw]
                    else:
                        src = ps[:, :nw]
                    nc.scalar.activation(
                        pb[:, c * 512 : c * 512 + nw],
                        src,
                        mybir.ActivationFunctionType.Exp,
                        accum_out=lparts[:, c : c + 1],
                    )
                lsum = small_pool.tile([P, 1], F32, name="lsum", tag="lsum")
                nc.vector.reduce_sum(lsum[:], lparts[:, :nchunks], axis=mybir.AxisListType.X)
                pt = pt_pool.tile([P, S], BF16, name="ptt", tag="ptt")
                for j in range(n):
                    tps = psum.tile([P, P], BF16, name="tps_t", tag="tps_t")
                    nc.tensor.transpose(tps[:], pb[:, j * P : (j + 1) * P], ident[:])
                    nc.vector.tensor_copy(pt[:, j * P : (j + 1) * P], tps[:])
                rl = small_pool.tile([P, 1], F32, name="rl", tag="rl")
                nc.vector.reciprocal(rl[:], lsum[:])
                ot = o_pool.tile([P, D], F32, name="ot", tag="ot")
                for ec in range(2):
                    ops = psum.tile([P, 512], F32, name="o_t", tag="o_t")
                    for j in range(n):
                        nc.tensor.matmul(
                            ops[:],
                            lhsT=pt[:, j * P : (j + 1) * P],
                            rhs=V[j][:, ec * 512 : (ec + 1) * 512],
                            start=(j == 0),
                            stop=(j == n - 1),
                        )
                    nc.vector.tensor_scalar_mul(ot[:, ec * 512 : (ec + 1) * 512], ops[:], rl[:])
                    nc.sync.dma_start(
                        out_d[s * P : (s + 1) * P, ec * 512 : (ec + 1) * 512],
                        ot[:, ec * 512 : (ec + 1) * 512],
                    )


def _get_nc():
    if "nc" not in _compiled:
        if MODE == "v1":
            _compiled["nc"] = _build_v1()
        else:
            _compiled["nc"] = _build_bf16()
    return _compiled["nc"]


def _host_mask(qg):
    karr = np.arange(S)
    m = np.zeros((1024, 512), np.float32)
    for s_i in range(8):
        kw = NS[s_i] * P
        nchunks = (kw + 511) // 512
        c0 = (nchunks - 1) * 512
        nw = kw - c0
        rows = qg[s_i * P : (s_i + 1) * P]
        m[s_i * P : (s_i + 1) * P, :nw] = np.where(
            karr[c0:kw][None, :] <= rows[:, None], np.float32(0.0), np.float32(MASK_VAL)
        )
    return m


def kernel(x, Wq, Wk, Wv):
    x = np.ascontiguousarray(np.asarray(x, dtype=np.float32))
    Wq = np.ascontiguousarray(np.asarray(Wq, dtype=np.float32))
    Wk = np.ascontiguousarray(np.asarray(Wk, dtype=np.float32))
    Wv = np.ascontiguousarray(np.asarray(Wv, dtype=np.float32))

    nc = _get_nc()
    bf = ml_dtypes.bfloat16

    Wq_c = np.ascontiguousarray(Wq.astype(bf))
    Wk_c = np.ascontiguousarray(Wk.astype(bf))
    WkT_c = np.ascontiguousarray(Wk.T.astype(bf))
    Wv_c = np.ascontiguousarray(Wv.astype(bf))
    in_maps = []
    for c in range(8):
        b, par = c // 2, c % 2
        tiles = A_TILES if par == 0 else B_TILES
        xb = x[b]
        xq = np.concatenate([xb[t * P : (t + 1) * P] for t in tiles], axis=0)
        xqT_np = np.ascontiguousarray((xq.T * np.float32(1.0 / 32.0)).astype(bf))
        xT_np = np.ascontiguousarray(xb.T.astype(bf))
        qg = np.concatenate([np.arange(t * P, (t + 1) * P) for t in tiles])
        m = _host_mask(qg)
        if MODE == "v1":
            im = {
                "xqT": xqT_np,
                "xT": xT_np,
                "xk": np.ascontiguousarray(xb.astype(bf)),
                "wq": Wq_c,
                "wkT": WkT_c,
                "wv": Wv_c,
                "msk": np.ascontiguousarray(m.astype(bf)),
            }
        else:
            im = {
                "xqT": xqT_np,
                "xT": xT_np,
                "wq": Wq_c,
                "wk": Wk_c,
                "wv": Wv_c,
                "msk": np.ascontiguousarray(m),
            }
        in_maps.append(im)

    trace = os.environ.get("BASS_KERNEL_TRACE", "0") == "1"
    res = run_bass_kernel_spmd(nc, in_maps, core_ids=list(range(8)), trace=trace)
    if trace:
        print(f"HW exec time: {res.exec_time_ns} ns")
        if res.instructions_and_trace is not None:
            print(f"trace: {res.instructions_and_trace[1]}")

    out = np.empty((B, S, D), np.float32)
    for c in range(8):
        b, par = c // 2, c % 2
        tiles = A_TILES if par == 0 else B_TILES
        o = res.results[c]["out"]
        for s_i, t in enumerate(tiles):
            out[b, t * P : (t + 1) * P] = o[s_i * P : (s_i + 1) * P]
    return out


# revision 18
# speedup vs baseline: 1.0864x; 1.0864x over previous
"""Causal self-attention (B=4, S=2048, D=1024, single head) on 8 TRN2 cores.

Sharding: data-parallel over batch (4 batches x 2 cores). The two cores of a
batch split the 16 query tiles (128 rows each) so both get exactly equal
causal work: core A takes tiles {0,2,4,6,9,11,13,15}, core B the complement.

v1 dataflow (default) eliminates the duplicated K/V projections entirely by
re-associating the attention algebra so that every projection-type matmul is
proportional to the LOCAL query count (1024 rows/core) instead of the full
key count (2048 keys, which both cores of a pair would otherwise each
project):

  scores = Q K^T = (Q Wk^T) X^T          -> Q'' = Q Wk^T is 1024-row local
  out    = P V   = (P X) Wv              -> P X is causal-sized; Wv-apply is
                                            1024-row local

Per core: Q^T projection (PE), Q''^T = Wk Q^T (PE), then per query-tile
slot: scores = Q''^T-lhsT vs X^T-rhs, exp on ACT with accumulated row sums,
P^T via plain matmuls against an identity rhs (cheaper than PE transpose
mode and HAM-warming), R^T = P X via pt-lhsT vs X-key-major rhs, R^T
re-transposed the same way, out = R^T Wv accumulated over d-chunks,
normalized by the reciprocal row sum. A block of warm-up matmuls on the
identity runs during the initial DMA window so the PE reaches its 2.4 GHz
clock (HAM K=8/8) before real work arrives.

Modes (BASS_KERNEL_MODE env, default "v1"):
  "v1"   — factored dataflow above; bf16 storage.
  "bf16" — previous baseline (duplicated K/V projections); ~233us.
"""

import os
from contextlib import ExitStack

import ml_dtypes
import numpy as np

import concourse.bacc as bacc
import concourse.mybir as mybir
import concourse.tile as tile
from concourse.bass_utils import run_bass_kernel_spmd

B, S, D = 4, 2048, 1024
P = 128
DC = D // P  # 8 contraction chunks
A_TILES = [0, 2, 4, 6, 9, 11, 13, 15]
B_TILES = [1, 3, 5, 7, 8, 10, 12, 14]
NS = [2, 4, 6, 8, 10, 12, 14, 16]  # key-tiles (128 keys) processed per slot
MASK_VAL = -60.0
N_WARM = 96

F32 = mybir.dt.float32
BF16 = mybir.dt.bfloat16

MODE = os.environ.get("BASS_KERNEL_MODE", "v1")

_compiled = {}


def _make_ident(tc, pool):
    nc = tc.nc
    ident = pool.tile([P, P], BF16, name="ident", tag="ident")
    nc.gpsimd.memset(ident[:], 1.0)
    nc.gpsimd.affine_select(
        out=ident[:],
        in_=ident[:],
        compare_op=mybir.AluOpType.is_equal,
        fill=0.0,
        base=0,
        pattern=[[-1, P]],
        channel_multiplier=1,
    )
    return ident


# ---------------------------------------------------------------------------
# v1: factored dataflow
# ---------------------------------------------------------------------------


def _build_v1():
    nc = bacc.Bacc("TRN2", target_bir_lowering=False, debug=False)
    xqT = nc.dram_tensor("xqT", [D, 1024], BF16, kind="ExternalInput").ap()
    xT = nc.dram_tensor("xT", [D, S], BF16, kind="ExternalInput").ap()
    xk = nc.dram_tensor("xk", [S, D], BF16, kind="ExternalInput").ap()
    wq = nc.dram_tensor("wq", [D, D], BF16, kind="ExternalInput").ap()
    wkT = nc.dram_tensor("wkT", [D, D], BF16, kind="ExternalInput").ap()
    wv = nc.dram_tensor("wv", [D, D], BF16, kind="ExternalInput").ap()
    msk = nc.dram_tensor("msk", [1024, 512], BF16, kind="ExternalInput").ap()
    out_d = nc.dram_tensor("out", [1024, D], F32, kind="ExternalOutput").ap()

    with tile.TileContext(nc) as tc:
        _body_v1(tc, xqT, xT, xk, wq, wkT, wv, msk, out_d)
    nc.compile()
    return nc


def _body_v1(tc, xqT, xT, xk, wq, wkT, wv, msk, out_d):
    nc = tc.nc
    with ExitStack() as top:
        const_pool = top.enter_context(tc.tile_pool(name="cst", bufs=1))
        ident = _make_ident(tc, const_pool)

        # whole-kernel residents
        res_pool = top.enter_context(tc.tile_pool(name="res", bufs=1))
        XT_t = [res_pool.tile([P, S], BF16, name=f"xt{d}", tag=f"xt{d}") for d in range(DC)]
        XK_t = [res_pool.tile([P, D], BF16, name=f"xk{k}", tag=f"xk{k}") for k in range(S // P)]
        Q2T = [res_pool.tile([P, 1024], BF16, name=f"q2t{d}", tag=f"q2t{d}") for d in range(DC)]
        wv_t = [res_pool.tile([P, D], BF16, name=f"wv{d}", tag=f"wv{d}") for d in range(DC)]

        # ---------------- projections ----------------
        with ExitStack() as ph:
            wq_pool = ph.enter_context(tc.tile_pool(name="wqp", bufs=1))
            xq_pool = ph.enter_context(tc.tile_pool(name="xqp", bufs=1))
            wkT_pool = ph.enter_context(tc.tile_pool(name="wkp", bufs=1))
            qt_pool = ph.enter_context(tc.tile_pool(name="qtp", bufs=1))
            pps = ph.enter_context(tc.tile_pool(name="pps", bufs=1, space="PSUM"))

            wq_t = [wq_pool.tile([P, D], BF16, name=f"wqt{d}", tag=f"wqt{d}") for d in range(DC)]
            xq_t = [xq_pool.tile([P, 1024], BF16, name=f"xq{d}", tag=f"xq{d}") for d in range(DC)]
            wkT_t = [wkT_pool.tile([P, D], BF16, name=f"wkt{e}", tag=f"wkt{e}") for e in range(DC)]
            QT = [qt_pool.tile([P, 1024], BF16, name=f"qt{e}", tag=f"qt{e}") for e in range(DC)]

            # DMA issue order = priority. Q-proj runs d-outer, so strip
            # PAIR d (wq_t[d] + xq_t[d], 0.5MB) is all the first d-sweep
            # needs — the PE starts ~3us in instead of waiting for 4MB.
            for d in range(DC):
                nc.sync.dma_start(wq_t[d][:], wq[d * P : (d + 1) * P, :])
                nc.sync.dma_start(xq_t[d][:], xqT[d * P : (d + 1) * P, :])
            for e in range(DC):
                nc.sync.dma_start(wkT_t[e][:], wkT[e * P : (e + 1) * P, :])
            for d in range(DC):
                nc.sync.dma_start(XT_t[d][:], xT[d * P : (d + 1) * P, :])
            for k in range(S // P):
                nc.sync.dma_start(XK_t[k][:], xk[k * P : (k + 1) * P, :])
            for d in range(DC):
                nc.sync.dma_start(wv_t[d][:], wv[d * P : (d + 1) * P, :])

            # Q^T projection, d-outer: 8 live psums (one per e-chunk) so the
            # d=0 sweep starts as soon as the first strip pair lands.
            for qc in range(2):
                qps = [pps.tile([P, 512], F32, name="pps", tag=f"pps{e}") for e in range(DC)]
                for d in range(DC):
                    for e in range(DC):
                        nc.tensor.matmul(
                            qps[e][:],
                            lhsT=wq_t[d][:, e * P : (e + 1) * P],
                            rhs=xq_t[d][:, qc * 512 : (qc + 1) * 512],
                            start=(d == 0),
                            stop=(d == DC - 1),
                        )
                for e in range(DC):
                    nc.scalar.copy(QT[e][:, qc * 512 : (qc + 1) * 512], qps[e][:])

            # Q''^T = Wk Q^T:  Q2T[dc][:, qc] = sum_e WkT[e, dc-chunk]^T QT[e, qc]
            for qc in range(2):
                for dc in range(DC):
                    ps = pps.tile([P, 512], F32, name="pps", tag=f"pps{dc}")
                    for e in range(DC):
                        nc.tensor.matmul(
                            ps[:],
                            lhsT=wkT_t[e][:, dc * P : (dc + 1) * P],
                            rhs=QT[e][:, qc * 512 : (qc + 1) * 512],
                            start=(e == 0),
                            stop=(e == DC - 1),
                        )
                    nc.scalar.copy(Q2T[dc][:, qc * 512 : (qc + 1) * 512], ps[:])

        # ---------------- attention over slots ----------------
        with ExitStack() as ph:
            m_pool = ph.enter_context(tc.tile_pool(name="mp", bufs=2))
            sm_pool = ph.enter_context(tc.tile_pool(name="smp", bufs=2))
            p_pool = ph.enter_context(tc.tile_pool(name="pp", bufs=2))
            pt_pool = ph.enter_context(tc.tile_pool(name="ptp", bufs=2))
            rt_pool = ph.enter_context(tc.tile_pool(name="rtp", bufs=2))
            r2_pool = ph.enter_context(tc.tile_pool(name="r2p", bufs=2))
            o_pool = ph.enter_context(tc.tile_pool(name="op", bufs=2))
            small_pool = ph.enter_context(tc.tile_pool(name="smallp", bufs=2))
            qk_ps = ph.enter_context(tc.tile_pool(name="qkps", bufs=2, space="PSUM"))
            tp_ps = ph.enter_context(tc.tile_pool(name="tpps", bufs=2, space="PSUM"))
            r_ps = ph.enter_context(tc.tile_pool(name="rps", bufs=1, space="PSUM"))
            o_ps = ph.enter_context(tc.tile_pool(name="ops", bufs=2, space="PSUM"))

            state = {}

            def scores_part(s):
                n = NS[s]
                kw = n * P
                nchunks = (kw + 511) // 512
                mt = m_pool.tile([P, 512], BF16, name="mt", tag="mt")
                last_nw = kw - (nchunks - 1) * 512
                nc.sync.dma_start(mt[:, :last_nw], msk[s * P : (s + 1) * P, :last_nw])
                pb = p_pool.tile([P, S], BF16, name="pb", tag="pb")
                lparts = small_pool.tile([P, 4], F32, name="lparts", tag="lparts")
                for c in range(nchunks):
                    nw = min(512, kw - c * 512)
                    ps = qk_ps.tile([P, 512], F32, name="qk_t", tag="qk_t")
                    for dc in range(DC):
                        nc.tensor.matmul(
                            ps[:, :nw],
                            lhsT=Q2T[dc][:, s * P : (s + 1) * P],
                            rhs=XT_t[dc][:, c * 512 : c * 512 + nw],
                            start=(dc == 0),
                            stop=(dc == DC - 1),
                        )
                    if c == nchunks - 1:
                        sm = sm_pool.tile([P, 512], F32, name="sm", tag="sm")
                        nc.vector.tensor_add(sm[:, :nw], ps[:, :nw], mt[:, :nw])
                        src = sm[:, :nw]
                    else:
                        src = ps[:, :nw]
                    nc.scalar.activation(
                        pb[:, c * 512 : c * 512 + nw],
                        src,
                        mybir.ActivationFunctionType.Exp,
                        accum_out=lparts[:, c : c + 1],
                    )
                lsum = small_pool.tile([P, 1], F32, name="lsum", tag="lsum")
                nc.vector.reduce_sum(lsum[:], lparts[:, :nchunks], axis=mybir.AxisListType.X)
                rl = small_pool.tile([P, 1], F32, name="rl", tag="rl")
                nc.vector.reciprocal(rl[:], lsum[:])
                state[s] = (pb, rl)

            def tail_a(s):
                n = NS[s]
                pb, rl = state[s]
                # P^T via plain matmuls: out = pb_block^T @ I, 4 tiles per
                # PSUM bank, one ACT copy per bank.
                pt = pt_pool.tile([P, S], BF16, name="ptt", tag="ptt")
                for j0 in range(0, n, 4):
                    jn = min(4, n - j0)
                    tp = tp_ps.tile([P, 512], F32, name="tp_t", tag="tp_t")
                    for j in range(j0, j0 + jn):
                        nc.tensor.matmul(
                            tp[:, (j - j0) * P : (j - j0 + 1) * P],
                            lhsT=pb[:, j * P : (j + 1) * P],
                            rhs=ident[:],
                            start=True,
                            stop=True,
                        )
                    nc.vector.tensor_copy(
                        pt[:, j0 * P : (j0 + jn) * P], tp[:, : jn * P]
                    )
                # R^T = P X  ([128q, 1024d], accumulated over n key tiles)
                rp = r_ps.tile([P, 1024], F32, name="rp", tag="rp")
                for j in range(n):
                    for h in range(2):
                        nc.tensor.matmul(
                            rp[:, h * 512 : (h + 1) * 512],
                            lhsT=pt[:, j * P : (j + 1) * P],
                            rhs=XK_t[j][:, h * 512 : (h + 1) * 512],
                            start=(j == 0),
                            stop=(j == n - 1),
                        )
                rt = rt_pool.tile([P, 1024], BF16, name="rt", tag="rt")
                for h in range(2):
                    nc.vector.tensor_copy(rt[:, h * 512 : (h + 1) * 512], rp[:, h * 512 : (h + 1) * 512])
                state[s] = (state[s][0], state[s][1], rt)

            def tail_b(s):
                pb, rl, rt = state.pop(s)
                # transpose R^T -> R chunks [128d, 128q]
                r2 = r2_pool.tile([P, 1024], BF16, name="r2", tag="r2")
                for d0 in range(0, DC, 4):
                    tp = tp_ps.tile([P, 512], F32, name="tp_t", tag="tp_t")
                    for dc in range(d0, d0 + 4):
                        nc.tensor.matmul(
                            tp[:, (dc - d0) * P : (dc - d0 + 1) * P],
                            lhsT=rt[:, dc * P : (dc + 1) * P],
                            rhs=ident[:],
                            start=True,
                            stop=True,
                        )
                    nc.vector.tensor_copy(r2[:, d0 * P : (d0 + 4) * P], tp[:])
                # out = R^T Wv  (accumulate over the 8 d-chunks)
                ops = [
                    o_ps.tile([P, 512], F32, name="o_t", tag="o_t"),
                    o_ps.tile([P, 512], F32, name="o_t", tag="o_t"),
                ]
                for dc in range(DC):
                    for h in range(2):
                        nc.tensor.matmul(
                            ops[h][:],
                            lhsT=r2[:, dc * P : (dc + 1) * P],
                            rhs=wv_t[dc][:, h * 512 : (h + 1) * 512],
                            start=(dc == 0),
                            stop=(dc == DC - 1),
                        )
                ot = o_pool.tile([P, D], F32, name="ot", tag="ot")
                for h in range(2):
                    nc.vector.tensor_scalar_mul(ot[:, h * 512 : (h + 1) * 512], ops[h][:], rl[:])
                    nc.sync.dma_start(
                        out_d[s * P : (s + 1) * P, h * 512 : (h + 1) * 512],
                        ot[:, h * 512 : (h + 1) * 512],
                    )

            # one-slot software pipeline: PE runs tail(s-1) while ACT
            # computes exp(s). The last two slots' tail halves interleave
            # so slot 7's handoffs (exp -> PT, rp -> rt) stay covered.
            for s in range(8):
                scores_part(s)
                if 1 <= s <= 6:
                    tail_a(s - 1)
                    tail_b(s - 1)
            tail_a(6)
            tail_a(7)
            tail_b(6)
            tail_b(7)


# ---------------------------------------------------------------------------
# previous baseline (duplicated K/V projections) — kept for A/B comparison
# ---------------------------------------------------------------------------


def _build_bf16():
    nc = bacc.Bacc("TRN2", target_bir_lowering=False, debug=False)
    xqT = nc.dram_tensor("xqT", [D, 1024], BF16, kind="ExternalInput").ap()
    xT = nc.dram_tensor("xT", [D, S], BF16, kind="ExternalInput").ap()
    wq = nc.dram_tensor("wq", [D, D], BF16, kind="ExternalInput").ap()
    wk = nc.dram_tensor("wk", [D, D], BF16, kind="ExternalInput").ap()
    wv = nc.dram_tensor("wv", [D, D], BF16, kind="ExternalInput").ap()
    msk = nc.dram_tensor("msk", [1024, 512], F32, kind="ExternalInput").ap()
    out_d = nc.dram_tensor("out", [1024, D], F32, kind="ExternalOutput").ap()

    with tile.TileContext(nc) as tc:
        _body_bf16(tc, xqT, xT, wq, wk, wv, msk, out_d)
    nc.compile()
    return nc


def _body_bf16(tc, xqT, xT, wq, wk, wv, msk, out_d):
    nc = tc.nc
    with ExitStack() as top:
        const_pool = top.enter_context(tc.tile_pool(name="cst", bufs=1))
        ident = _make_ident(tc, const_pool)

        res_pool = top.enter_context(tc.tile_pool(name="res", bufs=1))
        QT = [res_pool.tile([P, 1024], BF16, name=f"qt{e}", tag=f"qt{e}") for e in range(DC)]
        KT = [res_pool.tile([P, S], BF16, name=f"kt{e}", tag=f"kt{e}") for e in range(DC)]
        V = [res_pool.tile([P, D], BF16, name=f"v{k}", tag=f"v{k}") for k in range(S // P)]

        w_pool = top.enter_context(tc.tile_pool(name="wp", bufs=1))
        wq_t = [w_pool.tile([P, D], BF16, name=f"wqt{d}", tag=f"wqt{d}") for d in range(DC)]
        wk_t = [w_pool.tile([P, D], BF16, name=f"wkt{d}", tag=f"wkt{d}") for d in range(DC)]
        wv_t = [w_pool.tile([P, D], BF16, name=f"wvt{d}", tag=f"wvt{d}") for d in range(DC)]

        psum = top.enter_context(tc.tile_pool(name="psum", bufs=2, space="PSUM"))
        xs_pool = top.enter_context(tc.tile_pool(name="xsp", bufs=2))

        xs0 = [xs_pool.tile([P, 512], BF16, name=f"xs{d}", tag=f"xs{d}") for d in range(DC)]
        for d in range(DC):
            nc.sync.dma_start(xs0[d][:], xT[d * P : (d + 1) * P, 0:512])
        for ec in range(2):
            for d in range(DC):
                nc.sync.dma_start(
                    wv_t[d][:, ec * 512 : (ec + 1) * 512],
                    wv[d * P : (d + 1) * P, ec * 512 : (ec + 1) * 512],
                )
        for d in range(DC):
            nc.sync.dma_start(wk_t[d][:], wk[d * P : (d + 1) * P, :])
        for kc in range(4):
            if kc == 0:
                xs = xs0
            else:
                xs = [xs_pool.tile([P, 512], BF16, name=f"xs{d}", tag=f"xs{d}") for d in range(DC)]
                for d in range(DC):
                    nc.sync.dma_start(xs[d][:], xT[d * P : (d + 1) * P, kc * 512 : (kc + 1) * 512])
            for j in range(4):
                kt_idx = kc * 4 + j
                for ec in range(2):
                    ps = psum.tile([P, 512], F32, name="pps", tag="pps")
                    for d in range(DC):
                        nc.tensor.matmul(
                            ps[:],
                            lhsT=xs[d][:, j * P : (j + 1) * P],
                            rhs=wv_t[d][:, ec * 512 : (ec + 1) * 512],
                            start=(d == 0),
                            stop=(d == DC - 1),
                        )
                    nc.scalar.copy(V[kt_idx][:, ec * 512 : (ec + 1) * 512], ps[:])
            for e in range(DC):
                ps = psum.tile([P, 512], F32, name="pps", tag="pps")
                for d in range(DC):
                    nc.tensor.matmul(
                        ps[:],
                        lhsT=wk_t[d][:, e * P : (e + 1) * P],
                        rhs=xs[d][:],
                        start=(d == 0),
                        stop=(d == DC - 1),
                    )
                nc.scalar.copy(KT[e][:, kc * 512 : (kc + 1) * 512], ps[:])

        for d in range(DC):
            nc.sync.dma_start(wq_t[d][:], wq[d * P : (d + 1) * P, :])
        for qc in range(2):
            xs = [xs_pool.tile([P, 512], BF16, name=f"xs{d}", tag=f"xs{d}") for d in range(DC)]
            for d in range(DC):
                nc.sync.dma_start(xs[d][:], xqT[d * P : (d + 1) * P, qc * 512 : (qc + 1) * 512])
            for e in range(DC):
                ps = psum.tile([P, 512], F32, name="pps", tag="pps")
                for d in range(DC):
                    nc.tensor.matmul(
                        ps[:],
                        lhsT=wq_t[d][:, e * P : (e + 1) * P],
                        rhs=xs[d][:],
                        start=(d == 0),
                        stop=(d == DC - 1),
                    )
                nc.scalar.copy(QT[e][:, qc * 512 : (qc + 1) * 512], ps[:])

        with ExitStack() as ph:
            m_pool = ph.enter_context(tc.tile_pool(name="mp", bufs=2))
            sm_pool = ph.enter_context(tc.tile_pool(name="smp", bufs=2))
            p_pool = ph.enter_context(tc.tile_pool(name="pp", bufs=2))
            pt_pool = ph.enter_context(tc.tile_pool(name="ptp", bufs=2))
            o_pool = ph.enter_context(tc.tile_pool(name="op", bufs=2))
            small_pool = ph.enter_context(tc.tile_pool(name="smallp", bufs=2))

            for s in range(8):
                n = NS[s]
                kw = n * P
                nchunks = (kw + 511) // 512
                mt = m_pool.tile([P, 512], F32, name="mt", tag="mt")
                last_nw = kw - (nchunks - 1) * 512
                nc.sync.dma_start(mt[:, :last_nw], msk[s * P : (s + 1) * P, :last_nw])
                pb = p_pool.tile([P, S], BF16, name="pb", tag="pb")
                lparts = small_pool.tile([P, 4], F32, name="lparts", tag="lparts")
                for c in range(nchunks):
                    nw = min(512, kw - c * 512)
                    ps = psum.tile([P, 512], F32, name="qk_t", tag="qk_t")
                    for e in range(DC):
                        nc.tensor.matmul(
                            ps[:, :nw],
                            lhsT=QT[e][:, s * P : (s + 1) * P],
                            rhs=KT[e][:, c * 512 : c * 512 + nw],
                            start=(e == 0),
                            stop=(e == DC - 1),
                        )
                    if c == nchunks - 1:
                        sm = sm_pool.tile([P, 512], F32, name="sm", tag="sm")
                        nc.vector.tensor_add(sm[:, :nw], ps[:, :nw], mt[:, :nw])
                        src = sm[:, :nw]
                    else:
                        src = ps[:, :nw]
                    nc.scalar.activation(
                        pb[:, c * 512 : c * 512 + nw],
                        src,
                        mybir.ActivationFunctionType.Exp,
                        accum_out=lparts[:, c : c + 1],
                    )
                lsum = small_pool.tile([P, 1], F32, name="lsum", tag="lsum")
                nc.vector.reduce_sum(lsum[:], lparts[:, :nchunks], axis=mybir.AxisListType.X)
                pt = pt_pool.tile([P, S], BF16, name="ptt", tag="ptt")
                for j in range(n):
                    tps = psum.tile([P, P], BF16, name="tps_t", tag="tps_t")
                    nc.tensor.transpose(tps[:], pb[:, j * P : (j + 1) * P], ident[:])
                    nc.vector.tensor_copy(pt[:, j * P : (j + 1) * P], tps[:])
                rl = small_pool.tile([P, 1], F32, name="rl", tag="rl")
                nc.vector.reciprocal(rl[:], lsum[:])
                ot = o_pool.tile([P, D], F32, name="ot", tag="ot")
                for ec in range(2):
                    ops = psum.tile([P, 512], F32, name="o_t", tag="o_t")
                    for j in range(n):
                        nc.tensor.matmul(
                            ops[:],
                            lhsT=pt[:, j * P : (j + 1) * P],
                            rhs=V[j][:, ec * 512 : (ec + 1) * 512],
                            start=(j == 0),
                            stop=(j == n - 1),
                        )
                    nc.vector.tensor_scalar_mul(ot[:, ec * 512 : (ec + 1) * 512], ops[:], rl[:])
                    nc.sync.dma_start(
                        out_d[s * P : (s + 1) * P, ec * 512 : (ec + 1) * 512],
                        ot[:, ec * 512 : (ec + 1) * 512],
                    )


def _get_nc():
    if "nc" not in _compiled:
        if MODE == "v1":
            _compiled["nc"] = _build_v1()
        else:
            _compiled["nc"] = _build_bf16()
    return _compiled["nc"]


def _host_mask(qg):
    karr = np.arange(S)
    m = np.zeros((1024, 512), np.float32)
    for s_i in range(8):
        kw = NS[s_i] * P
        nchunks = (kw + 511) // 512
        c0 = (nchunks - 1) * 512
        nw = kw - c0
        rows = qg[s_i * P : (s_i + 1) * P]
        m[s_i * P : (s_i + 1) * P, :nw] = np.where(
            karr[c0:kw][None, :] <= rows[:, None], np.float32(0.0), np.float32(MASK_VAL)
        )
    return m


def kernel(x, Wq, Wk, Wv):
    x = np.ascontiguousarray(np.asarray(x, dtype=np.float32))
    Wq = np.ascontiguousarray(np.asarray(Wq, dtype=np.float32))
    Wk = np.ascontiguousarray(np.asarray(Wk, dtype=np.float32))
    Wv = np.ascontiguousarray(np.asarray(Wv, dtype=np.float32))

    nc = _get_nc()
    bf = ml_dtypes.bfloat16

    Wq_c = np.ascontiguousarray(Wq.astype(bf))
    Wk_c = np.ascontiguousarray(Wk.astype(bf))
    WkT_c = np.ascontiguousarray(Wk.T.astype(bf))
    Wv_c = np.ascontiguousarray(Wv.astype(bf))
    in_maps = []
    for c in range(8):
        b, par = c // 2, c % 2
        tiles = A_TILES if par == 0 else B_TILES
        xb = x[b]
        xq = np.concatenate([xb[t * P : (t + 1) * P] for t in tiles], axis=0)
        xqT_np = np.ascontiguousarray((xq.T * np.float32(1.0 / 32.0)).astype(bf))
        xT_np = np.ascontiguousarray(xb.T.astype(bf))
        qg = np.concatenate([np.arange(t * P, (t + 1) * P) for t in tiles])
        m = _host_mask(qg)
        if MODE == "v1":
            im = {
                "xqT": xqT_np,
                "xT": xT_np,
                "xk": np.ascontiguousarray(xb.astype(bf)),
                "wq": Wq_c,
                "wkT": WkT_c,
                "wv": Wv_c,
                "msk": np.ascontiguousarray(m.astype(bf)),
            }
        else:
            im = {
                "xqT": xqT_np,
                "xT": xT_np,
                "wq": Wq_c,
                "wk": Wk_c,
                "wv": Wv_c,
                "msk": np.ascontiguousarray(m),
            }
        in_maps.append(im)

    trace = os.environ.get("BASS_KERNEL_TRACE", "0") == "1"
    res = run_bass_kernel_spmd(nc, in_maps, core_ids=list(range(8)), trace=trace)
    if trace:
        print(f"HW exec time: {res.exec_time_ns} ns")
        if res.instructions_and_trace is not None:
            print(f"trace: {res.instructions_and_trace[1]}")

    out = np.empty((B, S, D), np.float32)
    for c in range(8):
        b, par = c // 2, c % 2
        tiles = A_TILES if par == 0 else B_TILES
        o = res.results[c]["out"]
        for s_i, t in enumerate(tiles):
            out[b, t * P : (t + 1) * P] = o[s_i * P : (s_i + 1) * P]
    return out


# revision 20
# speedup vs baseline: 1.0866x; 1.0002x over previous
"""Causal self-attention (B=4, S=2048, D=1024, single head) on 8 TRN2 cores.

Sharding: data-parallel over batch (4 batches x 2 cores). The two cores of a
batch split the 16 query tiles (128 rows each) so both get exactly equal
causal work: core A takes tiles {0,2,4,6,9,11,13,15}, core B the complement.

v1 dataflow (default) eliminates the duplicated K/V projections entirely by
re-associating the attention algebra so that every projection-type matmul is
proportional to the LOCAL query count (1024 rows/core) instead of the full
key count (2048 keys, which both cores of a pair would otherwise each
project):

  scores = Q K^T = (Q Wk^T) X^T          -> Q'' = Q Wk^T is 1024-row local
  out    = P V   = (P X) Wv              -> P X is causal-sized; Wv-apply is
                                            1024-row local

Per core: Q^T projection (PE), Q''^T = Wk Q^T (PE), then per query-tile
slot: scores = Q''^T-lhsT vs X^T-rhs, exp on ACT with accumulated row sums,
P^T via plain matmuls against an identity rhs (cheaper than PE transpose
mode and HAM-warming), R^T = P X via pt-lhsT vs X-key-major rhs, R^T
re-transposed the same way, out = R^T Wv accumulated over d-chunks,
normalized by the reciprocal row sum. A block of warm-up matmuls on the
identity runs during the initial DMA window so the PE reaches its 2.4 GHz
clock (HAM K=8/8) before real work arrives.

Modes (BASS_KERNEL_MODE env, default "v1"):
  "v1"   — factored dataflow above; bf16 storage.
  "bf16" — previous baseline (duplicated K/V projections); ~233us.
"""

import os
from contextlib import ExitStack

import ml_dtypes
import numpy as np

import concourse.bacc as bacc
import concourse.mybir as mybir
import concourse.tile as tile
from concourse.bass_utils import run_bass_kernel_spmd

B, S, D = 4, 2048, 1024
P = 128
DC = D // P  # 8 contraction chunks
A_TILES = [0, 2, 4, 6, 9, 11, 13, 15]
B_TILES = [1, 3, 5, 7, 8, 10, 12, 14]
NS = [2, 4, 6, 8, 10, 12, 14, 16]  # key-tiles (128 keys) processed per slot
MASK_VAL = -60.0
N_WARM = 96

F32 = mybir.dt.float32
BF16 = mybir.dt.bfloat16

MODE = os.environ.get("BASS_KERNEL_MODE", "v1")

_compiled = {}


def _make_ident(tc, pool):
    nc = tc.nc
    ident = pool.tile([P, P], BF16, name="ident", tag="ident")
    nc.gpsimd.memset(ident[:], 1.0)
    nc.gpsimd.affine_select(
        out=ident[:],
        in_=ident[:],
        compare_op=mybir.AluOpType.is_equal,
        fill=0.0,
        base=0,
        pattern=[[-1, P]],
        channel_multiplier=1,
    )
    return ident


# ---------------------------------------------------------------------------
# v1: factored dataflow
# ---------------------------------------------------------------------------


def _build_v1():
    nc = bacc.Bacc("TRN2", target_bir_lowering=False, debug=False)
    xqT = nc.dram_tensor("xqT", [D, 1024], BF16, kind="ExternalInput").ap()
    xT = nc.dram_tensor("xT", [D, S], BF16, kind="ExternalInput").ap()
    xk = nc.dram_tensor("xk", [S, D], BF16, kind="ExternalInput").ap()
    wq = nc.dram_tensor("wq", [D, D], BF16, kind="ExternalInput").ap()
    wkT = nc.dram_tensor("wkT", [D, D], BF16, kind="ExternalInput").ap()
    wv = nc.dram_tensor("wv", [D, D], BF16, kind="ExternalInput").ap()
    msk = nc.dram_tensor("msk", [1024, 512], BF16, kind="ExternalInput").ap()
    out_d = nc.dram_tensor("out", [1024, D], F32, kind="ExternalOutput").ap()

    with tile.TileContext(nc) as tc:
        _body_v1(tc, xqT, xT, xk, wq, wkT, wv, msk, out_d)
    nc.compile()
    return nc


def _body_v1(tc, xqT, xT, xk, wq, wkT, wv, msk, out_d):
    nc = tc.nc
    with ExitStack() as top:
        const_pool = top.enter_context(tc.tile_pool(name="cst", bufs=1))
        ident = _make_ident(tc, const_pool)

        # whole-kernel residents
        res_pool = top.enter_context(tc.tile_pool(name="res", bufs=1))
        XT_t = [res_pool.tile([P, S], BF16, name=f"xt{d}", tag=f"xt{d}") for d in range(DC)]
        XK_t = [res_pool.tile([P, D], BF16, name=f"xk{k}", tag=f"xk{k}") for k in range(S // P)]
        Q2T = [res_pool.tile([P, 1024], BF16, name=f"q2t{d}", tag=f"q2t{d}") for d in range(DC)]
        wv_t = [res_pool.tile([P, D], BF16, name=f"wv{d}", tag=f"wv{d}") for d in range(DC)]

        # ---------------- projections ----------------
        with ExitStack() as ph:
            wq_pool = ph.enter_context(tc.tile_pool(name="wqp", bufs=1))
            xq_pool = ph.enter_context(tc.tile_pool(name="xqp", bufs=1))
            wkT_pool = ph.enter_context(tc.tile_pool(name="wkp", bufs=1))
            qt_pool = ph.enter_context(tc.tile_pool(name="qtp", bufs=1))
            pps = ph.enter_context(tc.tile_pool(name="pps", bufs=1, space="PSUM"))

            wq_t = [wq_pool.tile([P, D], BF16, name=f"wqt{d}", tag=f"wqt{d}") for d in range(DC)]
            xq_t = [xq_pool.tile([P, 1024], BF16, name=f"xq{d}", tag=f"xq{d}") for d in range(DC)]
            wkT_t = [wkT_pool.tile([P, D], BF16, name=f"wkt{e}", tag=f"wkt{e}") for e in range(DC)]
            QT = [qt_pool.tile([P, 1024], BF16, name=f"qt{e}", tag=f"qt{e}") for e in range(DC)]

            # DMA issue order = priority. Q-proj runs d-outer, so strip
            # PAIR d (wq_t[d] + xq_t[d], 0.5MB) is all the first d-sweep
            # needs — the PE starts ~3us in instead of waiting for 4MB.
            for d in range(DC):
                nc.sync.dma_start(wq_t[d][:], wq[d * P : (d + 1) * P, :])
                nc.sync.dma_start(xq_t[d][:], xqT[d * P : (d + 1) * P, :])
            for e in range(DC):
                nc.sync.dma_start(wkT_t[e][:], wkT[e * P : (e + 1) * P, :])
            for d in range(DC):
                nc.sync.dma_start(XT_t[d][:], xT[d * P : (d + 1) * P, :])
            for k in range(S // P):
                nc.sync.dma_start(XK_t[k][:], xk[k * P : (k + 1) * P, :])
            for d in range(DC):
                nc.sync.dma_start(wv_t[d][:], wv[d * P : (d + 1) * P, :])

            # Q^T projection, d-outer: 8 live psums (one per e-chunk) so the
            # d=0 sweep starts as soon as the first strip pair lands.
            for qc in range(2):
                qps = [pps.tile([P, 512], F32, name="pps", tag=f"pps{e}") for e in range(DC)]
                for d in range(DC):
                    for e in range(DC):
                        nc.tensor.matmul(
                            qps[e][:],
                            lhsT=wq_t[d][:, e * P : (e + 1) * P],
                            rhs=xq_t[d][:, qc * 512 : (qc + 1) * 512],
                            start=(d == 0),
                            stop=(d == DC - 1),
                        )
                for e in range(DC):
                    nc.scalar.copy(QT[e][:, qc * 512 : (qc + 1) * 512], qps[e][:])

            # Q''^T = Wk Q^T:  Q2T[dc][:, qc] = sum_e WkT[e, dc-chunk]^T QT[e, qc]
            for qc in range(2):
                for dc in range(DC):
                    ps = pps.tile([P, 512], F32, name="pps", tag=f"pps{dc}")
                    for e in range(DC):
                        nc.tensor.matmul(
                            ps[:],
                            lhsT=wkT_t[e][:, dc * P : (dc + 1) * P],
                            rhs=QT[e][:, qc * 512 : (qc + 1) * 512],
                            start=(e == 0),
                            stop=(e == DC - 1),
                        )
                    nc.scalar.copy(Q2T[dc][:, qc * 512 : (qc + 1) * 512], ps[:])

        # ---------------- attention over slots ----------------
        with ExitStack() as ph:
            m_pool = ph.enter_context(tc.tile_pool(name="mp", bufs=2))
            sm_pool = ph.enter_context(tc.tile_pool(name="smp", bufs=2))
            p_pool = ph.enter_context(tc.tile_pool(name="pp", bufs=2))
            pt_pool = ph.enter_context(tc.tile_pool(name="ptp", bufs=2))
            rt_pool = ph.enter_context(tc.tile_pool(name="rtp", bufs=2))
            r2_pool = ph.enter_context(tc.tile_pool(name="r2p", bufs=2))
            o_pool = ph.enter_context(tc.tile_pool(name="op", bufs=2))
            small_pool = ph.enter_context(tc.tile_pool(name="smallp", bufs=2))
            qk_ps = ph.enter_context(tc.tile_pool(name="qkps", bufs=2, space="PSUM"))
            r_ps = ph.enter_context(tc.tile_pool(name="rps", bufs=2, space="PSUM"))
            o_ps = ph.enter_context(tc.tile_pool(name="ops", bufs=2, space="PSUM"))

            state = {}

            def scores_part(s):
                n = NS[s]
                kw = n * P
                nchunks = (kw + 511) // 512
                mt = m_pool.tile([P, 512], BF16, name="mt", tag="mt")
                last_nw = kw - (nchunks - 1) * 512
                nc.sync.dma_start(mt[:, :last_nw], msk[s * P : (s + 1) * P, :last_nw])
                pb = p_pool.tile([P, S], BF16, name="pb", tag="pb")
                lparts = small_pool.tile([P, 4], F32, name="lparts", tag="lparts")
                for c in range(nchunks):
                    nw = min(512, kw - c * 512)
                    ps = qk_ps.tile([P, 512], F32, name="qk_t", tag="qk_t")
                    for dc in range(DC):
                        nc.tensor.matmul(
                            ps[:, :nw],
                            lhsT=Q2T[dc][:, s * P : (s + 1) * P],
                            rhs=XT_t[dc][:, c * 512 : c * 512 + nw],
                            start=(dc == 0),
                            stop=(dc == DC - 1),
                        )
                    if c == nchunks - 1:
                        sm = sm_pool.tile([P, 512], F32, name="sm", tag="sm")
                        nc.vector.tensor_add(sm[:, :nw], ps[:, :nw], mt[:, :nw])
                        src = sm[:, :nw]
                    else:
                        src = ps[:, :nw]
                    nc.scalar.activation(
                        pb[:, c * 512 : c * 512 + nw],
                        src,
                        mybir.ActivationFunctionType.Exp,
                        accum_out=lparts[:, c : c + 1],
                    )
                lsum = small_pool.tile([P, 1], F32, name="lsum", tag="lsum")
                nc.vector.reduce_sum(lsum[:], lparts[:, :nchunks], axis=mybir.AxisListType.X)
                rl = small_pool.tile([P, 1], F32, name="rl", tag="rl")
                nc.vector.reciprocal(rl[:], lsum[:])
                state[s] = (pb, rl)

            def tail_a(s):
                n = NS[s]
                pb, rl = state[s]
                # P^T via one xbar DMA transpose: out[p, t, c] = pb[c, t*128+p]
                pt = pt_pool.tile([P, S // P, P], BF16, name="ptt", tag="ptt")
                nc.sync.dma_start_transpose(pt[:, 0:n, :], pb[:, 0 : n * P])
                # R^T = P X  ([128q, 1024d], accumulated over n key tiles)
                rp = r_ps.tile([P, 1024], F32, name="rp", tag="rp")
                for j in range(n):
                    for h in range(2):
                        nc.tensor.matmul(
                            rp[:, h * 512 : (h + 1) * 512],
                            lhsT=pt[:, j, :],
                            rhs=XK_t[j][:, h * 512 : (h + 1) * 512],
                            start=(j == 0),
                            stop=(j == n - 1),
                        )
                rt = rt_pool.tile([P, 1024], BF16, name="rt", tag="rt")
                for h in range(2):
                    nc.vector.tensor_copy(rt[:, h * 512 : (h + 1) * 512], rp[:, h * 512 : (h + 1) * 512])
                state[s] = (state[s][0], state[s][1], rt)

            def tail_b(s):
                pb, rl, rt = state.pop(s)
                # transpose R^T -> R chunks [128d, 128q] via xbar DMA
                r2 = r2_pool.tile([P, DC, P], BF16, name="r2", tag="r2")
                nc.sync.dma_start_transpose(r2[:, :, :], rt[:])
                # out = R^T Wv  (accumulate over the 8 d-chunks)
                ops = [
                    o_ps.tile([P, 512], F32, name="o_t", tag="o_t"),
                    o_ps.tile([P, 512], F32, name="o_t", tag="o_t"),
                ]
                for dc in range(DC):
                    for h in range(2):
                        nc.tensor.matmul(
                            ops[h][:],
                            lhsT=r2[:, dc, :],
                            rhs=wv_t[dc][:, h * 512 : (h + 1) * 512],
                            start=(dc == 0),
                            stop=(dc == DC - 1),
                        )
                ot = o_pool.tile([P, D], F32, name="ot", tag="ot")
                for h in range(2):
                    nc.vector.tensor_scalar_mul(ot[:, h * 512 : (h + 1) * 512], ops[h][:], rl[:])
                    nc.sync.dma_start(
                        out_d[s * P : (s + 1) * P, h * 512 : (h + 1) * 512],
                        ot[:, h * 512 : (h + 1) * 512],
                    )

            # one-slot software pipeline: PE runs tail(s-1) while ACT
            # computes exp(s). The last two slots' tail halves interleave
            # so slot 7's handoffs (exp -> PT, rp -> rt) stay covered.
            for s in range(8):
                scores_part(s)
                if 1 <= s <= 6:
                    tail_a(s - 1)
                    tail_b(s - 1)
            tail_a(6)
            tail_a(7)
            tail_b(6)
            tail_b(7)


# ---------------------------------------------------------------------------
# previous baseline (duplicated K/V projections) — kept for A/B comparison
# ---------------------------------------------------------------------------


def _build_bf16():
    nc = bacc.Bacc("TRN2", target_bir_lowering=False, debug=False)
    xqT = nc.dram_tensor("xqT", [D, 1024], BF16, kind="ExternalInput").ap()
    xT = nc.dram_tensor("xT", [D, S], BF16, kind="ExternalInput").ap()
    wq = nc.dram_tensor("wq", [D, D], BF16, kind="ExternalInput").ap()
    wk = nc.dram_tensor("wk", [D, D], BF16, kind="ExternalInput").ap()
    wv = nc.dram_tensor("wv", [D, D], BF16, kind="ExternalInput").ap()
    msk = nc.dram_tensor("msk", [1024, 512], F32, kind="ExternalInput").ap()
    out_d = nc.dram_tensor("out", [1024, D], F32, kind="ExternalOutput").ap()

    with tile.TileContext(nc) as tc:
        _body_bf16(tc, xqT, xT, wq, wk, wv, msk, out_d)
    nc.compile()
    return nc


def _body_bf16(tc, xqT, xT, wq, wk, wv, msk, out_d):
    nc = tc.nc
    with ExitStack() as top:
        const_pool = top.enter_context(tc.tile_pool(name="cst", bufs=1))
        ident = _make_ident(tc, const_pool)

        res_pool = top.enter_context(tc.tile_pool(name="res", bufs=1))
        QT = [res_pool.tile([P, 1024], BF16, name=f"qt{e}", tag=f"qt{e}") for e in range(DC)]
        KT = [res_pool.tile([P, S], BF16, name=f"kt{e}", tag=f"kt{e}") for e in range(DC)]
        V = [res_pool.tile([P, D], BF16, name=f"v{k}", tag=f"v{k}") for k in range(S // P)]

        w_pool = top.enter_context(tc.tile_pool(name="wp", bufs=1))
        wq_t = [w_pool.tile([P, D], BF16, name=f"wqt{d}", tag=f"wqt{d}") for d in range(DC)]
        wk_t = [w_pool.tile([P, D], BF16, name=f"wkt{d}", tag=f"wkt{d}") for d in range(DC)]
        wv_t = [w_pool.tile([P, D], BF16, name=f"wvt{d}", tag=f"wvt{d}") for d in range(DC)]

        psum = top.enter_context(tc.tile_pool(name="psum", bufs=2, space="PSUM"))
        xs_pool = top.enter_context(tc.tile_pool(name="xsp", bufs=2))

        xs0 = [xs_pool.tile([P, 512], BF16, name=f"xs{d}", tag=f"xs{d}") for d in range(DC)]
        for d in range(DC):
            nc.sync.dma_start(xs0[d][:], xT[d * P : (d + 1) * P, 0:512])
        for ec in range(2):
            for d in range(DC):
                nc.sync.dma_start(
                    wv_t[d][:, ec * 512 : (ec + 1) * 512],
                    wv[d * P : (d + 1) * P, ec * 512 : (ec + 1) * 512],
                )
        for d in range(DC):
            nc.sync.dma_start(wk_t[d][:], wk[d * P : (d + 1) * P, :])
        for kc in range(4):
            if kc == 0:
                xs = xs0
            else:
                xs = [xs_pool.tile([P, 512], BF16, name=f"xs{d}", tag=f"xs{d}") for d in range(DC)]
                for d in range(DC):
                    nc.sync.dma_start(xs[d][:], xT[d * P : (d + 1) * P, kc * 512 : (kc + 1) * 512])
            for j in range(4):
                kt_idx = kc * 4 + j
                for ec in range(2):
                    ps = psum.tile([P, 512], F32, name="pps", tag="pps")
                    for d in range(DC):
                        nc.tensor.matmul(
                            ps[:],
                            lhsT=xs[d][:, j * P : (j + 1) * P],
                            rhs=wv_t[d][:, ec * 512 : (ec + 1) * 512],
                            start=(d == 0),
                            stop=(d == DC - 1),
                        )
                    nc.scalar.copy(V[kt_idx][:, ec * 512 : (ec + 1) * 512], ps[:])
            for e in range(DC):
                ps = psum.tile([P, 512], F32, name="pps", tag="pps")
                for d in range(DC):
                    nc.tensor.matmul(
                        ps[:],
                        lhsT=wk_t[d][:, e * P : (e + 1) * P],
                        rhs=xs[d][:],
                        start=(d == 0),
                        stop=(d == DC - 1),
                    )
                nc.scalar.copy(KT[e][:, kc * 512 : (kc + 1) * 512], ps[:])

        for d in range(DC):
            nc.sync.dma_start(wq_t[d][:], wq[d * P : (d + 1) * P, :])
        for qc in range(2):
            xs = [xs_pool.tile([P, 512], BF16, name=f"xs{d}", tag=f"xs{d}") for d in range(DC)]
            for d in range(DC):
                nc.sync.dma_start(xs[d][:], xqT[d * P : (d + 1) * P, qc * 512 : (qc + 1) * 512])
            for e in range(DC):
                ps = psum.tile([P, 512], F32, name="pps", tag="pps")
                for d in range(DC):
                    nc.tensor.matmul(
                        ps[:],
                        lhsT=wq_t[d][:, e * P : (e + 1) * P],
                        rhs=xs[d][:],
                        start=(d == 0),
                        stop=(d == DC - 1),
                    )
                nc.scalar.copy(QT[e][:, qc * 512 : (qc + 1) * 512], ps[:])

        with ExitStack() as ph:
            m_pool = ph.enter_context(tc.tile_pool(name="mp", bufs=2))
            sm_pool = ph.enter_context(tc.tile_pool(name="smp", bufs=2))
            p_pool = ph.enter_context(tc.tile_pool(name="pp", bufs=2))
            pt_pool = ph.enter_context(tc.tile_pool(name="ptp", bufs=2))
            o_pool = ph.enter_context(tc.tile_pool(name="op", bufs=2))
            small_pool = ph.enter_context(tc.tile_pool(name="smallp", bufs=2))

            for s in range(8):
                n = NS[s]
                kw = n * P
                nchunks = (kw + 511) // 512
                mt = m_pool.tile([P, 512], F32, name="mt", tag="mt")
                last_nw = kw - (nchunks - 1) * 512
                nc.sync.dma_start(mt[:, :last_nw], msk[s * P : (s + 1) * P, :last_nw])
                pb = p_pool.tile([P, S], BF16, name="pb", tag="pb")
                lparts = small_pool.tile([P, 4], F32, name="lparts", tag="lparts")
                for c in range(nchunks):
                    nw = min(512, kw - c * 512)
                    ps = psum.tile([P, 512], F32, name="qk_t", tag="qk_t")
                    for e in range(DC):
                        nc.tensor.matmul(
                            ps[:, :nw],
                            lhsT=QT[e][:, s * P : (s + 1) * P],
                            rhs=KT[e][:, c * 512 : c * 512 + nw],
                            start=(e == 0),
                            stop=(e == DC - 1),
                        )
                    if c == nchunks - 1:
                        sm = sm_pool.tile([P, 512], F32, name="sm", tag="sm")
                        nc.vector.tensor_add(sm[:, :nw], ps[:, :nw], mt[:, :nw])
                        src = sm[:, :nw]
                    else:
                        src = ps[:, :nw]
                    nc.scalar.activation(
                        pb[:, c * 512 : c * 512 + nw],
                        src,
                        mybir.ActivationFunctionType.Exp,
                        accum_out=lparts[:, c : c + 1],
                    )
                lsum = small_pool.tile([P, 1], F32, name="lsum", tag="lsum")
                nc.vector.reduce_sum(lsum[:], lparts[:, :nchunks], axis=mybir.AxisListType.X)
                pt = pt_pool.tile([P, S], BF16, name="ptt", tag="ptt")
                for j in range(n):
                    tps = psum.tile([P, P], BF16, name="tps_t", tag="tps_t")
                    nc.tensor.transpose(tps[:], pb[:, j * P : (j + 1) * P], ident[:])
                    nc.vector.tensor_copy(pt[:, j * P : (j + 1) * P], tps[:])
                rl = small_pool.tile([P, 1], F32, name="rl", tag="rl")
                nc.vector.reciprocal(rl[:], lsum[:])
                ot = o_pool.tile([P, D], F32, name="ot", tag="ot")
                for ec in range(2):
                    ops = psum.tile([P, 512], F32, name="o_t", tag="o_t")
                    for j in range(n):
                        nc.tensor.matmul(
                            ops[:],
                            lhsT=pt[:, j * P : (j + 1) * P],
                            rhs=V[j][:, ec * 512 : (ec + 1) * 512],
                            start=(j == 0),
                            stop=(j == n - 1),
                        )
                    nc.vector.tensor_scalar_mul(ot[:, ec * 512 : (ec + 1) * 512], ops[:], rl[:])
                    nc.sync.dma_start(
                        out_d[s * P : (s + 1) * P, ec * 512 : (ec + 1) * 512],
                        ot[:, ec * 512 : (ec + 1) * 512],
                    )


def _get_nc():
    if "nc" not in _compiled:
        if MODE == "v1":
            _compiled["nc"] = _build_v1()
        else:
            _compiled["nc"] = _build_bf16()
    return _compiled["nc"]


def _host_mask(qg):
    karr = np.arange(S)
    m = np.zeros((1024, 512), np.float32)
    for s_i in range(8):
        kw = NS[s_i] * P
        nchunks = (kw + 511) // 512
        c0 = (nchunks - 1) * 512
        nw = kw - c0
        rows = qg[s_i * P : (s_i + 1) * P]
        m[s_i * P : (s_i + 1) * P, :nw] = np.where(
            karr[c0:kw][None, :] <= rows[:, None], np.float32(0.0), np.float32(MASK_VAL)
        )
    return m


def kernel(x, Wq, Wk, Wv):
    x = np.ascontiguousarray(np.asarray(x, dtype=np.float32))
    Wq = np.ascontiguousarray(np.asarray(Wq, dtype=np.float32))
    Wk = np.ascontiguousarray(np.asarray(Wk, dtype=np.float32))
    Wv = np.ascontiguousarray(np.asarray(Wv, dtype=np.float32))

    nc = _get_nc()
    bf = ml_dtypes.bfloat16

    Wq_c = np.ascontiguousarray(Wq.astype(bf))
    Wk_c = np.ascontiguousarray(Wk.astype(bf))
    WkT_c = np.ascontiguousarray(Wk.T.astype(bf))
    Wv_c = np.ascontiguousarray(Wv.astype(bf))
    in_maps = []
    for c in range(8):
        b, par = c // 2, c % 2
        tiles = A_TILES if par == 0 else B_TILES
        xb = x[b]
        xq = np.concatenate([xb[t * P : (t + 1) * P] for t in tiles], axis=0)
        xqT_np = np.ascontiguousarray((xq.T * np.float32(1.0 / 32.0)).astype(bf))
        xT_np = np.ascontiguousarray(xb.T.astype(bf))
        qg = np.concatenate([np.arange(t * P, (t + 1) * P) for t in tiles])
        m = _host_mask(qg)
        if MODE == "v1":
            im = {
                "xqT": xqT_np,
                "xT": xT_np,
                "xk": np.ascontiguousarray(xb.astype(bf)),
                "wq": Wq_c,
                "wkT": WkT_c,
                "wv": Wv_c,
                "msk": np.ascontiguousarray(m.astype(bf)),
            }
        else:
            im = {
                "xqT": xqT_np,
                "xT": xT_np,
                "wq": Wq_c,
                "wk": Wk_c,
                "wv": Wv_c,
                "msk": np.ascontiguousarray(m),
            }
        in_maps.append(im)

    trace = os.environ.get("BASS_KERNEL_TRACE", "0") == "1"
    res = run_bass_kernel_spmd(nc, in_maps, core_ids=list(range(8)), trace=trace)
    if trace:
        print(f"HW exec time: {res.exec_time_ns} ns")
        if res.instructions_and_trace is not None:
            print(f"trace: {res.instructions_and_trace[1]}")

    out = np.empty((B, S, D), np.float32)
    for c in range(8):
        b, par = c // 2, c % 2
        tiles = A_TILES if par == 0 else B_TILES
        o = res.results[c]["out"]
        for s_i, t in enumerate(tiles):
            out[b, t * P : (t + 1) * P] = o[s_i * P : (s_i + 1) * P]
    return out


# revision 21
# speedup vs baseline: 1.1159x; 1.0269x over previous
"""Causal self-attention (B=4, S=2048, D=1024, single head) on 8 TRN2 cores.

Sharding: data-parallel over batch (4 batches x 2 cores). The two cores of a
batch split the 16 query tiles (128 rows each) so both get exactly equal
causal work: core A takes tiles {0,2,4,6,9,11,13,15}, core B the complement.

v1 dataflow (default) eliminates the duplicated K/V projections entirely by
re-associating the attention algebra so that every projection-type matmul is
proportional to the LOCAL query count (1024 rows/core) instead of the full
key count (2048 keys, which both cores of a pair would otherwise each
project):

  scores = Q K^T = (Q Wk^T) X^T          -> Q'' = Q Wk^T is 1024-row local
  out    = P V   = (P X) Wv              -> P X is causal-sized; Wv-apply is
                                            1024-row local

Per core: Q^T projection (PE), Q''^T = Wk Q^T (PE), then per query-tile
slot: scores = Q''^T-lhsT vs X^T-rhs, exp on ACT with accumulated row sums,
P^T via plain matmuls against an identity rhs (cheaper than PE transpose
mode and HAM-warming), R^T = P X via pt-lhsT vs X-key-major rhs, R^T
re-transposed the same way, out = R^T Wv accumulated over d-chunks,
normalized by the reciprocal row sum. A block of warm-up matmuls on the
identity runs during the initial DMA window so the PE reaches its 2.4 GHz
clock (HAM K=8/8) before real work arrives.

Modes (BASS_KERNEL_MODE env, default "v1"):
  "v1"   — factored dataflow above; bf16 storage.
  "bf16" — previous baseline (duplicated K/V projections); ~233us.
"""

import os
from contextlib import ExitStack

import ml_dtypes
import numpy as np

import concourse.bacc as bacc
import concourse.mybir as mybir
import concourse.tile as tile
from concourse.bass_utils import run_bass_kernel_spmd

B, S, D = 4, 2048, 1024
P = 128
DC = D // P  # 8 contraction chunks
A_TILES = [0, 2, 4, 6, 9, 11, 13, 15]
B_TILES = [1, 3, 5, 7, 8, 10, 12, 14]
NS = [2, 4, 6, 8, 10, 12, 14, 16]  # key-tiles (128 keys) processed per slot
MASK_VAL = -60.0
N_WARM = 96

F32 = mybir.dt.float32
BF16 = mybir.dt.bfloat16

MODE = os.environ.get("BASS_KERNEL_MODE", "v1")

_compiled = {}


def _make_ident(tc, pool):
    nc = tc.nc
    ident = pool.tile([P, P], BF16, name="ident", tag="ident")
    nc.gpsimd.memset(ident[:], 1.0)
    nc.gpsimd.affine_select(
        out=ident[:],
        in_=ident[:],
        compare_op=mybir.AluOpType.is_equal,
        fill=0.0,
        base=0,
        pattern=[[-1, P]],
        channel_multiplier=1,
    )
    return ident


# ---------------------------------------------------------------------------
# v1: factored dataflow
# ---------------------------------------------------------------------------


def _build_v1():
    nc = bacc.Bacc("TRN2", target_bir_lowering=False, debug=False)
    xqT = nc.dram_tensor("xqT", [D, 1024], BF16, kind="ExternalInput").ap()
    xT = nc.dram_tensor("xT", [D, S], BF16, kind="ExternalInput").ap()
    xk = nc.dram_tensor("xk", [S, D], BF16, kind="ExternalInput").ap()
    wq = nc.dram_tensor("wq", [D, D], BF16, kind="ExternalInput").ap()
    wkT = nc.dram_tensor("wkT", [D, D], BF16, kind="ExternalInput").ap()
    wv = nc.dram_tensor("wv", [D, D], BF16, kind="ExternalInput").ap()
    msk = nc.dram_tensor("msk", [1024, 512], BF16, kind="ExternalInput").ap()
    out_d = nc.dram_tensor("out", [1024, D], F32, kind="ExternalOutput").ap()

    with tile.TileContext(nc) as tc:
        _body_v1(tc, xqT, xT, xk, wq, wkT, wv, msk, out_d)
    nc.compile()
    return nc


def _body_v1(tc, xqT, xT, xk, wq, wkT, wv, msk, out_d):
    nc = tc.nc
    with ExitStack() as top:
        const_pool = top.enter_context(tc.tile_pool(name="cst", bufs=1))
        ident = _make_ident(tc, const_pool)

        # whole-kernel residents
        res_pool = top.enter_context(tc.tile_pool(name="res", bufs=1))
        XT_t = [res_pool.tile([P, S], BF16, name=f"xt{d}", tag=f"xt{d}") for d in range(DC)]
        XK_t = [res_pool.tile([P, D], BF16, name=f"xk{k}", tag=f"xk{k}") for k in range(S // P)]
        Q2T = [res_pool.tile([P, 1024], BF16, name=f"q2t{d}", tag=f"q2t{d}") for d in range(DC)]
        wv_t = [res_pool.tile([P, D], BF16, name=f"wv{d}", tag=f"wv{d}") for d in range(DC)]

        # ---------------- projections ----------------
        with ExitStack() as ph:
            wq_pool = ph.enter_context(tc.tile_pool(name="wqp", bufs=1))
            xq_pool = ph.enter_context(tc.tile_pool(name="xqp", bufs=1))
            wkT_pool = ph.enter_context(tc.tile_pool(name="wkp", bufs=1))
            qt_pool = ph.enter_context(tc.tile_pool(name="qtp", bufs=1))
            pps = ph.enter_context(tc.tile_pool(name="pps", bufs=1, space="PSUM"))

            wq_t = [wq_pool.tile([P, D], BF16, name=f"wqt{d}", tag=f"wqt{d}") for d in range(DC)]
            xq_t = [xq_pool.tile([P, 1024], BF16, name=f"xq{d}", tag=f"xq{d}") for d in range(DC)]
            wkT_t = [wkT_pool.tile([P, D], BF16, name=f"wkt{e}", tag=f"wkt{e}") for e in range(DC)]
            QT = [qt_pool.tile([P, 1024], BF16, name=f"qt{e}", tag=f"qt{e}") for e in range(DC)]

            # DMA issue order = priority. Q-proj runs d-outer, so strip
            # PAIR d (wq_t[d] + xq_t[d], 0.5MB) is all the first d-sweep
            # needs — the PE starts ~3us in instead of waiting for 4MB.
            for d in range(DC):
                nc.sync.dma_start(wq_t[d][:], wq[d * P : (d + 1) * P, :])
                nc.sync.dma_start(xq_t[d][:], xqT[d * P : (d + 1) * P, :])
            for e in range(DC):
                nc.sync.dma_start(wkT_t[e][:], wkT[e * P : (e + 1) * P, :])
            for d in range(DC):
                nc.sync.dma_start(XT_t[d][:], xT[d * P : (d + 1) * P, :])
            for k in range(S // P):
                nc.sync.dma_start(XK_t[k][:], xk[k * P : (k + 1) * P, :])
            for d in range(DC):
                nc.sync.dma_start(wv_t[d][:], wv[d * P : (d + 1) * P, :])

            # Q^T projection, d-outer: 8 live psums (one per e-chunk) so the
            # d=0 sweep starts as soon as the first strip pair lands.
            for qc in range(2):
                qps = [pps.tile([P, 512], F32, name="pps", tag=f"pps{e}") for e in range(DC)]
                for d in range(DC):
                    for e in range(DC):
                        nc.tensor.matmul(
                            qps[e][:],
                            lhsT=wq_t[d][:, e * P : (e + 1) * P],
                            rhs=xq_t[d][:, qc * 512 : (qc + 1) * 512],
                            start=(d == 0),
                            stop=(d == DC - 1),
                        )
                for e in range(DC):
                    nc.scalar.copy(QT[e][:, qc * 512 : (qc + 1) * 512], qps[e][:])

            # Q''^T = Wk Q^T:  Q2T[dc][:, qc] = sum_e WkT[e, dc-chunk]^T QT[e, qc]
            for qc in range(2):
                for dc in range(DC):
                    ps = pps.tile([P, 512], F32, name="pps", tag=f"pps{dc}")
                    for e in range(DC):
                        nc.tensor.matmul(
                            ps[:],
                            lhsT=wkT_t[e][:, dc * P : (dc + 1) * P],
                            rhs=QT[e][:, qc * 512 : (qc + 1) * 512],
                            start=(e == 0),
                            stop=(e == DC - 1),
                        )
                    nc.scalar.copy(Q2T[dc][:, qc * 512 : (qc + 1) * 512], ps[:])

        # ---------------- attention over slots ----------------
        with ExitStack() as ph:
            m_pool = ph.enter_context(tc.tile_pool(name="mp", bufs=2))
            sm_pool = ph.enter_context(tc.tile_pool(name="smp", bufs=2))
            p_pool = ph.enter_context(tc.tile_pool(name="pp", bufs=2))
            pt_pool = ph.enter_context(tc.tile_pool(name="ptp", bufs=2))
            rt_pool = ph.enter_context(tc.tile_pool(name="rtp", bufs=2))
            r2_pool = ph.enter_context(tc.tile_pool(name="r2p", bufs=2))
            o_pool = ph.enter_context(tc.tile_pool(name="op", bufs=2))
            small_pool = ph.enter_context(tc.tile_pool(name="smallp", bufs=3))
            qk_ps = ph.enter_context(tc.tile_pool(name="qkps", bufs=2, space="PSUM"))
            r_ps = ph.enter_context(tc.tile_pool(name="rps", bufs=2, space="PSUM"))
            o_ps = ph.enter_context(tc.tile_pool(name="ops", bufs=2, space="PSUM"))

            state = {}

            def scores_part(s):
                n = NS[s]
                kw = n * P
                nchunks = (kw + 511) // 512
                mt = m_pool.tile([P, 512], BF16, name="mt", tag="mt")
                last_nw = kw - (nchunks - 1) * 512
                nc.sync.dma_start(mt[:, :last_nw], msk[s * P : (s + 1) * P, :last_nw])
                pb = p_pool.tile([P, S], BF16, name="pb", tag="pb")
                lparts = small_pool.tile([P, 4], F32, name="lparts", tag="lparts")
                for c in range(nchunks):
                    nw = min(512, kw - c * 512)
                    ps = qk_ps.tile([P, 512], F32, name="qk_t", tag="qk_t")
                    for dc in range(DC):
                        nc.tensor.matmul(
                            ps[:, :nw],
                            lhsT=Q2T[dc][:, s * P : (s + 1) * P],
                            rhs=XT_t[dc][:, c * 512 : c * 512 + nw],
                            start=(dc == 0),
                            stop=(dc == DC - 1),
                        )
                    if c == nchunks - 1:
                        sm = sm_pool.tile([P, 512], F32, name="sm", tag="sm")
                        nc.vector.tensor_add(sm[:, :nw], ps[:, :nw], mt[:, :nw])
                        src = sm[:, :nw]
                    else:
                        src = ps[:, :nw]
                    nc.scalar.activation(
                        pb[:, c * 512 : c * 512 + nw],
                        src,
                        mybir.ActivationFunctionType.Exp,
                        accum_out=lparts[:, c : c + 1],
                    )
                lsum = small_pool.tile([P, 1], F32, name="lsum", tag="lsum")
                nc.vector.reduce_sum(lsum[:], lparts[:, :nchunks], axis=mybir.AxisListType.X)
                rl = small_pool.tile([P, 1], F32, name="rl", tag="rl")
                nc.vector.reciprocal(rl[:], lsum[:])
                # issue the P^T xbar transpose here so its DMA latency is
                # covered by the next slot's score matmuls
                pt = pt_pool.tile([P, S // P, P], BF16, name="ptt", tag="ptt")
                nc.sync.dma_start_transpose(pt[:, 0:n, :], pb[:, 0 : n * P])
                state[s] = (pt, rl)

            def tail_a(s):
                n = NS[s]
                pt, rl = state[s]
                # R^T = P X  ([128q, 1024d], accumulated over n key tiles)
                rp = r_ps.tile([P, 1024], F32, name="rp", tag="rp")
                for j in range(n):
                    for h in range(2):
                        nc.tensor.matmul(
                            rp[:, h * 512 : (h + 1) * 512],
                            lhsT=pt[:, j, :],
                            rhs=XK_t[j][:, h * 512 : (h + 1) * 512],
                            start=(j == 0),
                            stop=(j == n - 1),
                        )
                rt = rt_pool.tile([P, 1024], BF16, name="rt", tag="rt")
                for h in range(2):
                    nc.vector.tensor_copy(rt[:, h * 512 : (h + 1) * 512], rp[:, h * 512 : (h + 1) * 512])
                # R^T -> R chunks [128d, 128q] via xbar DMA, issued here so
                # the latency is covered before tail_b consumes r2
                r2 = r2_pool.tile([P, DC, P], BF16, name="r2", tag="r2")
                nc.sync.dma_start_transpose(r2[:, :, :], rt[:])
                state[s] = (state[s][0], state[s][1], r2)

            def tail_b(s):
                pt, rl, r2 = state.pop(s)
                # out = R^T Wv  (accumulate over the 8 d-chunks)
                ops = [
                    o_ps.tile([P, 512], F32, name="o_t", tag="o_t"),
                    o_ps.tile([P, 512], F32, name="o_t", tag="o_t"),
                ]
                for dc in range(DC):
                    for h in range(2):
                        nc.tensor.matmul(
                            ops[h][:],
                            lhsT=r2[:, dc, :],
                            rhs=wv_t[dc][:, h * 512 : (h + 1) * 512],
                            start=(dc == 0),
                            stop=(dc == DC - 1),
                        )
                ot = o_pool.tile([P, D], F32, name="ot", tag="ot")
                for h in range(2):
                    nc.vector.tensor_scalar_mul(ot[:, h * 512 : (h + 1) * 512], ops[h][:], rl[:])
                    nc.sync.dma_start(
                        out_d[s * P : (s + 1) * P, h * 512 : (h + 1) * 512],
                        ot[:, h * 512 : (h + 1) * 512],
                    )

            # two-stage software pipeline: tail_b lags tail_a by one slot
            # so every serial handoff (exp -> P^T DMA -> R, rp -> rt -> R^T
            # DMA -> apply) is covered by a neighbouring slot's PE work.
            for s in range(8):
                scores_part(s)
                if s >= 1:
                    tail_a(s - 1)
                if s >= 2:
                    tail_b(s - 2)
            tail_a(7)
            tail_b(6)
            tail_b(7)


# ---------------------------------------------------------------------------
# previous baseline (duplicated K/V projections) — kept for A/B comparison
# ---------------------------------------------------------------------------


def _build_bf16():
    nc = bacc.Bacc("TRN2", target_bir_lowering=False, debug=False)
    xqT = nc.dram_tensor("xqT", [D, 1024], BF16, kind="ExternalInput").ap()
    xT = nc.dram_tensor("xT", [D, S], BF16, kind="ExternalInput").ap()
    wq = nc.dram_tensor("wq", [D, D], BF16, kind="ExternalInput").ap()
    wk = nc.dram_tensor("wk", [D, D], BF16, kind="ExternalInput").ap()
    wv = nc.dram_tensor("wv", [D, D], BF16, kind="ExternalInput").ap()
    msk = nc.dram_tensor("msk", [1024, 512], F32, kind="ExternalInput").ap()
    out_d = nc.dram_tensor("out", [1024, D], F32, kind="ExternalOutput").ap()

    with tile.TileContext(nc) as tc:
        _body_bf16(tc, xqT, xT, wq, wk, wv, msk, out_d)
    nc.compile()
    return nc


def _body_bf16(tc, xqT, xT, wq, wk, wv, msk, out_d):
    nc = tc.nc
    with ExitStack() as top:
        const_pool = top.enter_context(tc.tile_pool(name="cst", bufs=1))
        ident = _make_ident(tc, const_pool)

        res_pool = top.enter_context(tc.tile_pool(name="res", bufs=1))
        QT = [res_pool.tile([P, 1024], BF16, name=f"qt{e}", tag=f"qt{e}") for e in range(DC)]
        KT = [res_pool.tile([P, S], BF16, name=f"kt{e}", tag=f"kt{e}") for e in range(DC)]
        V = [res_pool.tile([P, D], BF16, name=f"v{k}", tag=f"v{k}") for k in range(S // P)]

        w_pool = top.enter_context(tc.tile_pool(name="wp", bufs=1))
        wq_t = [w_pool.tile([P, D], BF16, name=f"wqt{d}", tag=f"wqt{d}") for d in range(DC)]
        wk_t = [w_pool.tile([P, D], BF16, name=f"wkt{d}", tag=f"wkt{d}") for d in range(DC)]
        wv_t = [w_pool.tile([P, D], BF16, name=f"wvt{d}", tag=f"wvt{d}") for d in range(DC)]

        psum = top.enter_context(tc.tile_pool(name="psum", bufs=2, space="PSUM"))
        xs_pool = top.enter_context(tc.tile_pool(name="xsp", bufs=2))

        xs0 = [xs_pool.tile([P, 512], BF16, name=f"xs{d}", tag=f"xs{d}") for d in range(DC)]
        for d in range(DC):
            nc.sync.dma_start(xs0[d][:], xT[d * P : (d + 1) * P, 0:512])
        for ec in range(2):
            for d in range(DC):
                nc.sync.dma_start(
                    wv_t[d][:, ec * 512 : (ec + 1) * 512],
                    wv[d * P : (d + 1) * P, ec * 512 : (ec + 1) * 512],
                )
        for d in range(DC):
            nc.sync.dma_start(wk_t[d][:], wk[d * P : (d + 1) * P, :])
        for kc in range(4):
            if kc == 0:
                xs = xs0
            else:
                xs = [xs_pool.tile([P, 512], BF16, name=f"xs{d}", tag=f"xs{d}") for d in range(DC)]
                for d in range(DC):
                    nc.sync.dma_start(xs[d][:], xT[d * P : (d + 1) * P, kc * 512 : (kc + 1) * 512])
            for j in range(4):
                kt_idx = kc * 4 + j
                for ec in range(2):
                    ps = psum.tile([P, 512], F32, name="pps", tag="pps")
                    for d in range(DC):
                        nc.tensor.matmul(
                            ps[:],
                            lhsT=xs[d][:, j * P : (j + 1) * P],
                            rhs=wv_t[d][:, ec * 512 : (ec + 1) * 512],
                            start=(d == 0),
                            stop=(d == DC - 1),
                        )
                    nc.scalar.copy(V[kt_idx][:, ec * 512 : (ec + 1) * 512], ps[:])
            for e in range(DC):
                ps = psum.tile([P, 512], F32, name="pps", tag="pps")
                for d in range(DC):
                    nc.tensor.matmul(
                        ps[:],
                        lhsT=wk_t[d][:, e * P : (e + 1) * P],
                        rhs=xs[d][:],
                        start=(d == 0),
                        stop=(d == DC - 1),
                    )
                nc.scalar.copy(KT[e][:, kc * 512 : (kc + 1) * 512], ps[:])

        for d in range(DC):
            nc.sync.dma_start(wq_t[d][:], wq[d * P : (d + 1) * P, :])
        for qc in range(2):
            xs = [xs_pool.tile([P, 512], BF16, name=f"xs{d}", tag=f"xs{d}") for d in range(DC)]
            for d in range(DC):
                nc.sync.dma_start(xs[d][:], xqT[d * P : (d + 1) * P, qc * 512 : (qc + 1) * 512])
            for e in range(DC):
                ps = psum.tile([P, 512], F32, name="pps", tag="pps")
                for d in range(DC):
                    nc.tensor.matmul(
                        ps[:],
                        lhsT=wq_t[d][:, e * P : (e + 1) * P],
                        rhs=xs[d][:],
                        start=(d == 0),
                        stop=(d == DC - 1),
                    )
                nc.scalar.copy(QT[e][:, qc * 512 : (qc + 1) * 512], ps[:])

        with ExitStack() as ph:
            m_pool = ph.enter_context(tc.tile_pool(name="mp", bufs=2))
            sm_pool = ph.enter_context(tc.tile_pool(name="smp", bufs=2))
            p_pool = ph.enter_context(tc.tile_pool(name="pp", bufs=2))
            pt_pool = ph.enter_context(tc.tile_pool(name="ptp", bufs=2))
            o_pool = ph.enter_context(tc.tile_pool(name="op", bufs=2))
            small_pool = ph.enter_context(tc.tile_pool(name="smallp", bufs=3))

            for s in range(8):
                n = NS[s]
                kw = n * P
                nchunks = (kw + 511) // 512
                mt = m_pool.tile([P, 512], F32, name="mt", tag="mt")
                last_nw = kw - (nchunks - 1) * 512
                nc.sync.dma_start(mt[:, :last_nw], msk[s * P : (s + 1) * P, :last_nw])
                pb = p_pool.tile([P, S], BF16, name="pb", tag="pb")
                lparts = small_pool.tile([P, 4], F32, name="lparts", tag="lparts")
                for c in range(nchunks):
                    nw = min(512, kw - c * 512)
                    ps = psum.tile([P, 512], F32, name="qk_t", tag="qk_t")
                    for e in range(DC):
                        nc.tensor.matmul(
                            ps[:, :nw],
                            lhsT=QT[e][:, s * P : (s + 1) * P],
                            rhs=KT[e][:, c * 512 : c * 512 + nw],
                            start=(e == 0),
                            stop=(e == DC - 1),
                        )
                    if c == nchunks - 1:
                        sm = sm_pool.tile([P, 512], F32, name="sm", tag="sm")
                        nc.vector.tensor_add(sm[:, :nw], ps[:, :nw], mt[:, :nw])
                        src = sm[:, :nw]
                    else:
                        src = ps[:, :nw]
                    nc.scalar.activation(
                        pb[:, c * 512 : c * 512 + nw],
                        src,
                        mybir.ActivationFunctionType.Exp,
                        accum_out=lparts[:, c : c + 1],
                    )
                lsum = small_pool.tile([P, 1], F32, name="lsum", tag="lsum")
                nc.vector.reduce_sum(lsum[:], lparts[:, :nchunks], axis=mybir.AxisListType.X)
                pt = pt_pool.tile([P, S], BF16, name="ptt", tag="ptt")
                for j in range(n):
                    tps = psum.tile([P, P], BF16, name="tps_t", tag="tps_t")
                    nc.tensor.transpose(tps[:], pb[:, j * P : (j + 1) * P], ident[:])
                    nc.vector.tensor_copy(pt[:, j * P : (j + 1) * P], tps[:])
                rl = small_pool.tile([P, 1], F32, name="rl", tag="rl")
                nc.vector.reciprocal(rl[:], lsum[:])
                ot = o_pool.tile([P, D], F32, name="ot", tag="ot")
                for ec in range(2):
                    ops = psum.tile([P, 512], F32, name="o_t", tag="o_t")
                    for j in range(n):
                        nc.tensor.matmul(
                            ops[:],
                            lhsT=pt[:, j * P : (j + 1) * P],
                            rhs=V[j][:, ec * 512 : (ec + 1) * 512],
                            start=(j == 0),
                            stop=(j == n - 1),
                        )
                    nc.vector.tensor_scalar_mul(ot[:, ec * 512 : (ec + 1) * 512], ops[:], rl[:])
                    nc.sync.dma_start(
                        out_d[s * P : (s + 1) * P, ec * 512 : (ec + 1) * 512],
                        ot[:, ec * 512 : (ec + 1) * 512],
                    )


def _get_nc():
    if "nc" not in _compiled:
        if MODE == "v1":
            _compiled["nc"] = _build_v1()
        else:
            _compiled["nc"] = _build_bf16()
    return _compiled["nc"]


def _host_mask(qg):
    karr = np.arange(S)
    m = np.zeros((1024, 512), np.float32)
    for s_i in range(8):
        kw = NS[s_i] * P
        nchunks = (kw + 511) // 512
        c0 = (nchunks - 1) * 512
        nw = kw - c0
        rows = qg[s_i * P : (s_i + 1) * P]
        m[s_i * P : (s_i + 1) * P, :nw] = np.where(
            karr[c0:kw][None, :] <= rows[:, None], np.float32(0.0), np.float32(MASK_VAL)
        )
    return m


def kernel(x, Wq, Wk, Wv):
    x = np.ascontiguousarray(np.asarray(x, dtype=np.float32))
    Wq = np.ascontiguousarray(np.asarray(Wq, dtype=np.float32))
    Wk = np.ascontiguousarray(np.asarray(Wk, dtype=np.float32))
    Wv = np.ascontiguousarray(np.asarray(Wv, dtype=np.float32))

    nc = _get_nc()
    bf = ml_dtypes.bfloat16

    Wq_c = np.ascontiguousarray(Wq.astype(bf))
    Wk_c = np.ascontiguousarray(Wk.astype(bf))
    WkT_c = np.ascontiguousarray(Wk.T.astype(bf))
    Wv_c = np.ascontiguousarray(Wv.astype(bf))
    in_maps = []
    for c in range(8):
        b, par = c // 2, c % 2
        tiles = A_TILES if par == 0 else B_TILES
        xb = x[b]
        xq = np.concatenate([xb[t * P : (t + 1) * P] for t in tiles], axis=0)
        xqT_np = np.ascontiguousarray((xq.T * np.float32(1.0 / 32.0)).astype(bf))
        xT_np = np.ascontiguousarray(xb.T.astype(bf))
        qg = np.concatenate([np.arange(t * P, (t + 1) * P) for t in tiles])
        m = _host_mask(qg)
        if MODE == "v1":
            im = {
                "xqT": xqT_np,
                "xT": xT_np,
                "xk": np.ascontiguousarray(xb.astype(bf)),
                "wq": Wq_c,
                "wkT": WkT_c,
                "wv": Wv_c,
                "msk": np.ascontiguousarray(m.astype(bf)),
            }
        else:
            im = {
                "xqT": xqT_np,
                "xT": xT_np,
                "wq": Wq_c,
                "wk": Wk_c,
                "wv": Wv_c,
                "msk": np.ascontiguousarray(m),
            }
        in_maps.append(im)

    trace = os.environ.get("BASS_KERNEL_TRACE", "0") == "1"
    res = run_bass_kernel_spmd(nc, in_maps, core_ids=list(range(8)), trace=trace)
    if trace:
        print(f"HW exec time: {res.exec_time_ns} ns")
        if res.instructions_and_trace is not None:
            print(f"trace: {res.instructions_and_trace[1]}")

    out = np.empty((B, S, D), np.float32)
    for c in range(8):
        b, par = c // 2, c % 2
        tiles = A_TILES if par == 0 else B_TILES
        o = res.results[c]["out"]
        for s_i, t in enumerate(tiles):
            out[b, t * P : (t + 1) * P] = o[s_i * P : (s_i + 1) * P]
    return out


# revision 22
# speedup vs baseline: 1.1173x; 1.0013x over previous
"""Causal self-attention (B=4, S=2048, D=1024, single head) on 8 TRN2 cores.

Sharding: data-parallel over batch (4 batches x 2 cores). The two cores of a
batch split the 16 query tiles (128 rows each) so both get exactly equal
causal work: core A takes tiles {0,2,4,6,9,11,13,15}, core B the complement.

v1 dataflow (default) eliminates the duplicated K/V projections entirely by
re-associating the attention algebra so that every projection-type matmul is
proportional to the LOCAL query count (1024 rows/core) instead of the full
key count (2048 keys, which both cores of a pair would otherwise each
project):

  scores = Q K^T = (Q Wk^T) X^T          -> Q'' = Q Wk^T is 1024-row local
  out    = P V   = (P X) Wv              -> P X is causal-sized; Wv-apply is
                                            1024-row local

Per core: Q^T projection (PE), Q''^T = Wk Q^T (PE), then per query-tile
slot: scores = Q''^T-lhsT vs X^T-rhs, exp on ACT with accumulated row sums,
P^T via plain matmuls against an identity rhs (cheaper than PE transpose
mode and HAM-warming), R^T = P X via pt-lhsT vs X-key-major rhs, R^T
re-transposed the same way, out = R^T Wv accumulated over d-chunks,
normalized by the reciprocal row sum. A block of warm-up matmuls on the
identity runs during the initial DMA window so the PE reaches its 2.4 GHz
clock (HAM K=8/8) before real work arrives.

Modes (BASS_KERNEL_MODE env, default "v1"):
  "v1"   — factored dataflow above; bf16 storage.
  "bf16" — previous baseline (duplicated K/V projections); ~233us.
"""

import os
from contextlib import ExitStack

import ml_dtypes
import numpy as np

import concourse.bacc as bacc
import concourse.mybir as mybir
import concourse.tile as tile
from concourse.bass_utils import run_bass_kernel_spmd

B, S, D = 4, 2048, 1024
P = 128
DC = D // P  # 8 contraction chunks
A_TILES = [0, 2, 4, 6, 9, 11, 13, 15]
B_TILES = [1, 3, 5, 7, 8, 10, 12, 14]
NS = [2, 4, 6, 8, 10, 12, 14, 16]  # key-tiles (128 keys) processed per slot
MASK_VAL = -60.0
N_WARM = 96

F32 = mybir.dt.float32
BF16 = mybir.dt.bfloat16

MODE = os.environ.get("BASS_KERNEL_MODE", "v1")

_compiled = {}


def _make_ident(tc, pool):
    nc = tc.nc
    ident = pool.tile([P, P], BF16, name="ident", tag="ident")
    nc.gpsimd.memset(ident[:], 1.0)
    nc.gpsimd.affine_select(
        out=ident[:],
        in_=ident[:],
        compare_op=mybir.AluOpType.is_equal,
        fill=0.0,
        base=0,
        pattern=[[-1, P]],
        channel_multiplier=1,
    )
    return ident


# ---------------------------------------------------------------------------
# v1: factored dataflow
# ---------------------------------------------------------------------------


def _build_v1():
    nc = bacc.Bacc("TRN2", target_bir_lowering=False, debug=False)
    xqT = nc.dram_tensor("xqT", [D, 1024], BF16, kind="ExternalInput").ap()
    xT = nc.dram_tensor("xT", [D, S], BF16, kind="ExternalInput").ap()
    xk = nc.dram_tensor("xk", [S, D], BF16, kind="ExternalInput").ap()
    wq = nc.dram_tensor("wq", [D, D], BF16, kind="ExternalInput").ap()
    wkT = nc.dram_tensor("wkT", [D, D], BF16, kind="ExternalInput").ap()
    wv = nc.dram_tensor("wv", [D, D], BF16, kind="ExternalInput").ap()
    msk = nc.dram_tensor("msk", [1024, 512], BF16, kind="ExternalInput").ap()
    out_d = nc.dram_tensor("out", [1024, D], F32, kind="ExternalOutput").ap()

    with tile.TileContext(nc) as tc:
        _body_v1(tc, xqT, xT, xk, wq, wkT, wv, msk, out_d)
    nc.compile()
    return nc


def _body_v1(tc, xqT, xT, xk, wq, wkT, wv, msk, out_d):
    nc = tc.nc
    with ExitStack() as top:
        const_pool = top.enter_context(tc.tile_pool(name="cst", bufs=1))
        ident = _make_ident(tc, const_pool)

        # whole-kernel residents
        res_pool = top.enter_context(tc.tile_pool(name="res", bufs=1))
        XT_t = [res_pool.tile([P, S], BF16, name=f"xt{d}", tag=f"xt{d}") for d in range(DC)]
        XK_t = [res_pool.tile([P, D], BF16, name=f"xk{k}", tag=f"xk{k}") for k in range(S // P)]
        Q2T = [res_pool.tile([P, 1024], BF16, name=f"q2t{d}", tag=f"q2t{d}") for d in range(DC)]
        wv_t = [res_pool.tile([P, D], BF16, name=f"wv{d}", tag=f"wv{d}") for d in range(DC)]

        # ---------------- projections ----------------
        with ExitStack() as ph:
            wq_pool = ph.enter_context(tc.tile_pool(name="wqp", bufs=1))
            xq_pool = ph.enter_context(tc.tile_pool(name="xqp", bufs=1))
            wkT_pool = ph.enter_context(tc.tile_pool(name="wkp", bufs=1))
            qt_pool = ph.enter_context(tc.tile_pool(name="qtp", bufs=1))
            pps = ph.enter_context(tc.tile_pool(name="pps", bufs=1, space="PSUM"))

            wq_t = [wq_pool.tile([P, D], BF16, name=f"wqt{d}", tag=f"wqt{d}") for d in range(DC)]
            xq_t = [xq_pool.tile([P, 1024], BF16, name=f"xq{d}", tag=f"xq{d}") for d in range(DC)]
            wkT_t = [wkT_pool.tile([P, D], BF16, name=f"wkt{e}", tag=f"wkt{e}") for e in range(DC)]
            QT = [qt_pool.tile([P, 1024], BF16, name=f"qt{e}", tag=f"qt{e}") for e in range(DC)]

            # DMA issue order = priority. Q-proj runs d-outer, so strip
            # PAIR d (wq_t[d] + xq_t[d], 0.5MB) is all the first d-sweep
            # needs — the PE starts ~3us in instead of waiting for 4MB.
            for d in range(DC):
                nc.sync.dma_start(wq_t[d][:], wq[d * P : (d + 1) * P, :])
                nc.sync.dma_start(xq_t[d][:], xqT[d * P : (d + 1) * P, :])
            for e in range(DC):
                nc.sync.dma_start(wkT_t[e][:], wkT[e * P : (e + 1) * P, :])
            for d in range(DC):
                nc.sync.dma_start(XT_t[d][:], xT[d * P : (d + 1) * P, :])
            for k in range(S // P):
                nc.sync.dma_start(XK_t[k][:], xk[k * P : (k + 1) * P, :])
            for d in range(DC):
                nc.sync.dma_start(wv_t[d][:], wv[d * P : (d + 1) * P, :])

            # Q^T projection, d-outer: 8 live psums (one per e-chunk) so the
            # d=0 sweep starts as soon as the first strip pair lands.
            for qc in range(2):
                qps = [pps.tile([P, 512], F32, name="pps", tag=f"pps{e}") for e in range(DC)]
                for d in range(DC):
                    for e in range(DC):
                        nc.tensor.matmul(
                            qps[e][:],
                            lhsT=wq_t[d][:, e * P : (e + 1) * P],
                            rhs=xq_t[d][:, qc * 512 : (qc + 1) * 512],
                            start=(d == 0),
                            stop=(d == DC - 1),
                        )
                for e in range(DC):
                    nc.scalar.copy(QT[e][:, qc * 512 : (qc + 1) * 512], qps[e][:])

            # Q''^T = Wk Q^T:  Q2T[dc][:, qc] = sum_e WkT[e, dc-chunk]^T QT[e, qc]
            for qc in range(2):
                for dc in range(DC):
                    ps = pps.tile([P, 512], F32, name="pps", tag=f"pps{dc}")
                    for e in range(DC):
                        nc.tensor.matmul(
                            ps[:],
                            lhsT=wkT_t[e][:, dc * P : (dc + 1) * P],
                            rhs=QT[e][:, qc * 512 : (qc + 1) * 512],
                            start=(e == 0),
                            stop=(e == DC - 1),
                        )
                    nc.scalar.copy(Q2T[dc][:, qc * 512 : (qc + 1) * 512], ps[:])

        # ---------------- attention over slots ----------------
        with ExitStack() as ph:
            m_pool = ph.enter_context(tc.tile_pool(name="mp", bufs=2))
            sm_pool = ph.enter_context(tc.tile_pool(name="smp", bufs=2))
            p_pool = ph.enter_context(tc.tile_pool(name="pp", bufs=2))
            pt_pool = ph.enter_context(tc.tile_pool(name="ptp", bufs=2))
            rt_pool = ph.enter_context(tc.tile_pool(name="rtp", bufs=2))
            r2_pool = ph.enter_context(tc.tile_pool(name="r2p", bufs=2))
            o_pool = ph.enter_context(tc.tile_pool(name="op", bufs=2))
            small_pool = ph.enter_context(tc.tile_pool(name="smallp", bufs=3))
            qk_ps = ph.enter_context(tc.tile_pool(name="qkps", bufs=2, space="PSUM"))
            r_ps = ph.enter_context(tc.tile_pool(name="rps", bufs=2, space="PSUM"))
            o_ps = ph.enter_context(tc.tile_pool(name="ops", bufs=2, space="PSUM"))

            state = {}

            def scores_part(s):
                n = NS[s]
                kw = n * P
                nchunks = (kw + 511) // 512
                mt = m_pool.tile([P, 512], BF16, name="mt", tag="mt")
                last_nw = kw - (nchunks - 1) * 512
                nc.sync.dma_start(mt[:, :last_nw], msk[s * P : (s + 1) * P, :last_nw])
                pb = p_pool.tile([P, S], BF16, name="pb", tag="pb")
                lparts = small_pool.tile([P, 4], F32, name="lparts", tag="lparts")
                for c in range(nchunks):
                    nw = min(512, kw - c * 512)
                    ps = qk_ps.tile([P, 512], F32, name="qk_t", tag="qk_t")
                    for dc in range(DC):
                        nc.tensor.matmul(
                            ps[:, :nw],
                            lhsT=Q2T[dc][:, s * P : (s + 1) * P],
                            rhs=XT_t[dc][:, c * 512 : c * 512 + nw],
                            start=(dc == 0),
                            stop=(dc == DC - 1),
                        )
                    if c == nchunks - 1:
                        sm = sm_pool.tile([P, 512], F32, name="sm", tag="sm")
                        nc.vector.tensor_add(sm[:, :nw], ps[:, :nw], mt[:, :nw])
                        src = sm[:, :nw]
                    else:
                        src = ps[:, :nw]
                    nc.scalar.activation(
                        pb[:, c * 512 : c * 512 + nw],
                        src,
                        mybir.ActivationFunctionType.Exp,
                        accum_out=lparts[:, c : c + 1],
                    )
                lsum = small_pool.tile([P, 1], F32, name="lsum", tag="lsum")
                nc.vector.reduce_sum(lsum[:], lparts[:, :nchunks], axis=mybir.AxisListType.X)
                rl = small_pool.tile([P, 1], F32, name="rl", tag="rl")
                nc.vector.reciprocal(rl[:], lsum[:])
                # issue the P^T xbar transpose here so its DMA latency is
                # covered by the next slot's score matmuls
                pt = pt_pool.tile([P, S // P, P], BF16, name="ptt", tag="ptt")
                nc.sync.dma_start_transpose(pt[:, 0:n, :], pb[:, 0 : n * P])
                state[s] = (pt, rl)

            def tail_a(s):
                n = NS[s]
                pt, rl = state[s]
                # R^T = P X  ([128q, 1024d], accumulated over n key tiles)
                rp = r_ps.tile([P, 1024], F32, name="rp", tag="rp")
                for j in range(n):
                    for h in range(2):
                        nc.tensor.matmul(
                            rp[:, h * 512 : (h + 1) * 512],
                            lhsT=pt[:, j, :],
                            rhs=XK_t[j][:, h * 512 : (h + 1) * 512],
                            start=(j == 0),
                            stop=(j == n - 1),
                        )
                rt = rt_pool.tile([P, 1024], BF16, name="rt", tag="rt")
                for h in range(2):
                    nc.vector.tensor_copy(rt[:, h * 512 : (h + 1) * 512], rp[:, h * 512 : (h + 1) * 512])
                # R^T -> R chunks [128d, 128q] via xbar DMA, issued here so
                # the latency is covered before tail_b consumes r2
                r2 = r2_pool.tile([P, DC, P], BF16, name="r2", tag="r2")
                nc.sync.dma_start_transpose(r2[:, :, :], rt[:])
                state[s] = (state[s][0], state[s][1], r2)

            def tail_b(s):
                pt, rl, r2 = state.pop(s)
                # out = R^T Wv  (accumulate over the 8 d-chunks)
                ops = [
                    o_ps.tile([P, 512], F32, name="o_t", tag="o_t"),
                    o_ps.tile([P, 512], F32, name="o_t", tag="o_t"),
                ]
                for dc in range(DC):
                    for h in range(2):
                        nc.tensor.matmul(
                            ops[h][:],
                            lhsT=r2[:, dc, :],
                            rhs=wv_t[dc][:, h * 512 : (h + 1) * 512],
                            start=(dc == 0),
                            stop=(dc == DC - 1),
                        )
                ot = o_pool.tile([P, D], F32, name="ot", tag="ot")
                for h in range(2):
                    nc.vector.tensor_scalar_mul(ot[:, h * 512 : (h + 1) * 512], ops[h][:], rl[:])
                    nc.sync.dma_start(
                        out_d[s * P : (s + 1) * P, h * 512 : (h + 1) * 512],
                        ot[:, h * 512 : (h + 1) * 512],
                    )

            # two-stage software pipeline: tail_b lags tail_a by one slot
            # so every serial handoff (exp -> P^T DMA -> R, rp -> rt -> R^T
            # DMA -> apply) is covered by a neighbouring slot's PE work.
            order = [7, 6, 5, 4, 3, 2, 1, 0]
            for k, s in enumerate(order):
                scores_part(s)
                if k >= 1:
                    tail_a(order[k - 1])
                if k >= 2:
                    tail_b(order[k - 2])
            tail_a(order[7])
            tail_b(order[6])
            tail_b(order[7])


# ---------------------------------------------------------------------------
# previous baseline (duplicated K/V projections) — kept for A/B comparison
# ---------------------------------------------------------------------------


def _build_bf16():
    nc = bacc.Bacc("TRN2", target_bir_lowering=False, debug=False)
    xqT = nc.dram_tensor("xqT", [D, 1024], BF16, kind="ExternalInput").ap()
    xT = nc.dram_tensor("xT", [D, S], BF16, kind="ExternalInput").ap()
    wq = nc.dram_tensor("wq", [D, D], BF16, kind="ExternalInput").ap()
    wk = nc.dram_tensor("wk", [D, D], BF16, kind="ExternalInput").ap()
    wv = nc.dram_tensor("wv", [D, D], BF16, kind="ExternalInput").ap()
    msk = nc.dram_tensor("msk", [1024, 512], F32, kind="ExternalInput").ap()
    out_d = nc.dram_tensor("out", [1024, D], F32, kind="ExternalOutput").ap()

    with tile.TileContext(nc) as tc:
        _body_bf16(tc, xqT, xT, wq, wk, wv, msk, out_d)
    nc.compile()
    return nc


def _body_bf16(tc, xqT, xT, wq, wk, wv, msk, out_d):
    nc = tc.nc
    with ExitStack() as top:
        const_pool = top.enter_context(tc.tile_pool(name="cst", bufs=1))
        ident = _make_ident(tc, const_pool)

        res_pool = top.enter_context(tc.tile_pool(name="res", bufs=1))
        QT = [res_pool.tile([P, 1024], BF16, name=f"qt{e}", tag=f"qt{e}") for e in range(DC)]
        KT = [res_pool.tile([P, S], BF16, name=f"kt{e}", tag=f"kt{e}") for e in range(DC)]
        V = [res_pool.tile([P, D], BF16, name=f"v{k}", tag=f"v{k}") for k in range(S // P)]

        w_pool = top.enter_context(tc.tile_pool(name="wp", bufs=1))
        wq_t = [w_pool.tile([P, D], BF16, name=f"wqt{d}", tag=f"wqt{d}") for d in range(DC)]
        wk_t = [w_pool.tile([P, D], BF16, name=f"wkt{d}", tag=f"wkt{d}") for d in range(DC)]
        wv_t = [w_pool.tile([P, D], BF16, name=f"wvt{d}", tag=f"wvt{d}") for d in range(DC)]

        psum = top.enter_context(tc.tile_pool(name="psum", bufs=2, space="PSUM"))
        xs_pool = top.enter_context(tc.tile_pool(name="xsp", bufs=2))

        xs0 = [xs_pool.tile([P, 512], BF16, name=f"xs{d}", tag=f"xs{d}") for d in range(DC)]
        for d in range(DC):
            nc.sync.dma_start(xs0[d][:], xT[d * P : (d + 1) * P, 0:512])
        for ec in range(2):
            for d in range(DC):
                nc.sync.dma_start(
                    wv_t[d][:, ec * 512 : (ec + 1) * 512],
                    wv[d * P : (d + 1) * P, ec * 512 : (ec + 1) * 512],
                )
        for d in range(DC):
            nc.sync.dma_start(wk_t[d][:], wk[d * P : (d + 1) * P, :])
        for kc in range(4):
            if kc == 0:
                xs = xs0
            else:
                xs = [xs_pool.tile([P, 512], BF16, name=f"xs{d}", tag=f"xs{d}") for d in range(DC)]
                for d in range(DC):
                    nc.sync.dma_start(xs[d][:], xT[d * P : (d + 1) * P, kc * 512 : (kc + 1) * 512])
            for j in range(4):
                kt_idx = kc * 4 + j
                for ec in range(2):
                    ps = psum.tile([P, 512], F32, name="pps", tag="pps")
                    for d in range(DC):
                        nc.tensor.matmul(
                            ps[:],
                            lhsT=xs[d][:, j * P : (j + 1) * P],
                            rhs=wv_t[d][:, ec * 512 : (ec + 1) * 512],
                            start=(d == 0),
                            stop=(d == DC - 1),
                        )
                    nc.scalar.copy(V[kt_idx][:, ec * 512 : (ec + 1) * 512], ps[:])
            for e in range(DC):
                ps = psum.tile([P, 512], F32, name="pps", tag="pps")
                for d in range(DC):
                    nc.tensor.matmul(
                        ps[:],
                        lhsT=wk_t[d][:, e * P : (e + 1) * P],
                        rhs=xs[d][:],
                        start=(d == 0),
                        stop=(d == DC - 1),
                    )
                nc.scalar.copy(KT[e][:, kc * 512 : (kc + 1) * 512], ps[:])

        for d in range(DC):
            nc.sync.dma_start(wq_t[d][:], wq[d * P : (d + 1) * P, :])
        for qc in range(2):
            xs = [xs_pool.tile([P, 512], BF16, name=f"xs{d}", tag=f"xs{d}") for d in range(DC)]
            for d in range(DC):
                nc.sync.dma_start(xs[d][:], xqT[d * P : (d + 1) * P, qc * 512 : (qc + 1) * 512])
            for e in range(DC):
                ps = psum.tile([P, 512], F32, name="pps", tag="pps")
                for d in range(DC):
                    nc.tensor.matmul(
                        ps[:],
                        lhsT=wq_t[d][:, e * P : (e + 1) * P],
                        rhs=xs[d][:],
                        start=(d == 0),
                        stop=(d == DC - 1),
                    )
                nc.scalar.copy(QT[e][:, qc * 512 : (qc + 1) * 512], ps[:])

        with ExitStack() as ph:
            m_pool = ph.enter_context(tc.tile_pool(name="mp", bufs=2))
            sm_pool = ph.enter_context(tc.tile_pool(name="smp", bufs=2))
            p_pool = ph.enter_context(tc.tile_pool(name="pp", bufs=2))
            pt_pool = ph.enter_context(tc.tile_pool(name="ptp", bufs=2))
            o_pool = ph.enter_context(tc.tile_pool(name="op", bufs=2))
            small_pool = ph.enter_context(tc.tile_pool(name="smallp", bufs=3))

            for s in range(8):
                n = NS[s]
                kw = n * P
                nchunks = (kw + 511) // 512
                mt = m_pool.tile([P, 512], F32, name="mt", tag="mt")
                last_nw = kw - (nchunks - 1) * 512
                nc.sync.dma_start(mt[:, :last_nw], msk[s * P : (s + 1) * P, :last_nw])
                pb = p_pool.tile([P, S], BF16, name="pb", tag="pb")
                lparts = small_pool.tile([P, 4], F32, name="lparts", tag="lparts")
                for c in range(nchunks):
                    nw = min(512, kw - c * 512)
                    ps = psum.tile([P, 512], F32, name="qk_t", tag="qk_t")
                    for e in range(DC):
                        nc.tensor.matmul(
                            ps[:, :nw],
                            lhsT=QT[e][:, s * P : (s + 1) * P],
                            rhs=KT[e][:, c * 512 : c * 512 + nw],
                            start=(e == 0),
                            stop=(e == DC - 1),
                        )
                    if c == nchunks - 1:
                        sm = sm_pool.tile([P, 512], F32, name="sm", tag="sm")
                        nc.vector.tensor_add(sm[:, :nw], ps[:, :nw], mt[:, :nw])
                        src = sm[:, :nw]
                    else:
                        src = ps[:, :nw]
                    nc.scalar.activation(
                        pb[:, c * 512 : c * 512 + nw],
                        src,
                        mybir.ActivationFunctionType.Exp,
                        accum_out=lparts[:, c : c + 1],
                    )
                lsum = small_pool.tile([P, 1], F32, name="lsum", tag="lsum")
                nc.vector.reduce_sum(lsum[:], lparts[:, :nchunks], axis=mybir.AxisListType.X)
                pt = pt_pool.tile([P, S], BF16, name="ptt", tag="ptt")
                for j in range(n):
                    tps = psum.tile([P, P], BF16, name="tps_t", tag="tps_t")
                    nc.tensor.transpose(tps[:], pb[:, j * P : (j + 1) * P], ident[:])
                    nc.vector.tensor_copy(pt[:, j * P : (j + 1) * P], tps[:])
                rl = small_pool.tile([P, 1], F32, name="rl", tag="rl")
                nc.vector.reciprocal(rl[:], lsum[:])
                ot = o_pool.tile([P, D], F32, name="ot", tag="ot")
                for ec in range(2):
                    ops = psum.tile([P, 512], F32, name="o_t", tag="o_t")
                    for j in range(n):
                        nc.tensor.matmul(
                            ops[:],
                            lhsT=pt[:, j * P : (j + 1) * P],
                            rhs=V[j][:, ec * 512 : (ec + 1) * 512],
                            start=(j == 0),
                            stop=(j == n - 1),
                        )
                    nc.vector.tensor_scalar_mul(ot[:, ec * 512 : (ec + 1) * 512], ops[:], rl[:])
                    nc.sync.dma_start(
                        out_d[s * P : (s + 1) * P, ec * 512 : (ec + 1) * 512],
                        ot[:, ec * 512 : (ec + 1) * 512],
                    )


def _get_nc():
    if "nc" not in _compiled:
        if MODE == "v1":
            _compiled["nc"] = _build_v1()
        else:
            _compiled["nc"] = _build_bf16()
    return _compiled["nc"]


def _host_mask(qg):
    karr = np.arange(S)
    m = np.zeros((1024, 512), np.float32)
    for s_i in range(8):
        kw = NS[s_i] * P
        nchunks = (kw + 511) // 512
        c0 = (nchunks - 1) * 512
        nw = kw - c0
        rows = qg[s_i * P : (s_i + 1) * P]
        m[s_i * P : (s_i + 1) * P, :nw] = np.where(
            karr[c0:kw][None, :] <= rows[:, None], np.float32(0.0), np.float32(MASK_VAL)
        )
    return m


def kernel(x, Wq, Wk, Wv):
    x = np.ascontiguousarray(np.asarray(x, dtype=np.float32))
    Wq = np.ascontiguousarray(np.asarray(Wq, dtype=np.float32))
    Wk = np.ascontiguousarray(np.asarray(Wk, dtype=np.float32))
    Wv = np.ascontiguousarray(np.asarray(Wv, dtype=np.float32))

    nc = _get_nc()
    bf = ml_dtypes.bfloat16

    Wq_c = np.ascontiguousarray(Wq.astype(bf))
    Wk_c = np.ascontiguousarray(Wk.astype(bf))
    WkT_c = np.ascontiguousarray(Wk.T.astype(bf))
    Wv_c = np.ascontiguousarray(Wv.astype(bf))
    in_maps = []
    for c in range(8):
        b, par = c // 2, c % 2
        tiles = A_TILES if par == 0 else B_TILES
        xb = x[b]
        xq = np.concatenate([xb[t * P : (t + 1) * P] for t in tiles], axis=0)
        xqT_np = np.ascontiguousarray((xq.T * np.float32(1.0 / 32.0)).astype(bf))
        xT_np = np.ascontiguousarray(xb.T.astype(bf))
        qg = np.concatenate([np.arange(t * P, (t + 1) * P) for t in tiles])
        m = _host_mask(qg)
        if MODE == "v1":
            im = {
                "xqT": xqT_np,
                "xT": xT_np,
                "xk": np.ascontiguousarray(xb.astype(bf)),
                "wq": Wq_c,
                "wkT": WkT_c,
                "wv": Wv_c,
                "msk": np.ascontiguousarray(m.astype(bf)),
            }
        else:
            im = {
                "xqT": xqT_np,
                "xT": xT_np,
                "wq": Wq_c,
                "wk": Wk_c,
                "wv": Wv_c,
                "msk": np.ascontiguousarray(m),
            }
        in_maps.append(im)

    trace = os.environ.get("BASS_KERNEL_TRACE", "0") == "1"
    res = run_bass_kernel_spmd(nc, in_maps, core_ids=list(range(8)), trace=trace)
    if trace:
        print(f"HW exec time: {res.exec_time_ns} ns")
        if res.instructions_and_trace is not None:
            print(f"trace: {res.instructions_and_trace[1]}")

    out = np.empty((B, S, D), np.float32)
    for c in range(8):
        b, par = c // 2, c % 2
        tiles = A_TILES if par == 0 else B_TILES
        o = res.results[c]["out"]
        for s_i, t in enumerate(tiles):
            out[b, t * P : (t + 1) * P] = o[s_i * P : (s_i + 1) * P]
    return out
